# revision 2
# baseline (speedup 1.0000x reference)
"""Llama GQA attention (B=1, Q=1024, PAST=3072, HID=4096, NH=32, NKV=8, HD=128)
tensor-parallel over heads across 8 NeuronCores.

Per core c: kv head c, query heads 4c..4c+3. Each core computes its partial
o_proj contribution [1024, 4096] in bf16; the host sums the 8 partials in f32.

v2 layout (vs v1): engineered to unload the DVE (v1 bottleneck: 441us busy).
  - q/k proj W-stationary: out is [d, seq] (born transposed, no PE transposes).
    RoPE rotate-half via a PE permutation matmul; combine = 3 DVE TT ops.
  - v proj hs-stationary: out is [seq, d] directly in attn lhsT layout.
  - scores land in bf16 PSUM supertiles [128, 2048] (2 kv tiles x 1024 q);
    exp is ONE fused ACTIVATE per supertile reading PSUM directly. No mask
    add except on the diagonal tiles (DVE, in-place in PSUM).
  - causal skip: kv tiles 28..31 only computed against queries 512..1023
    (one extra narrow-quad supertile); kv 24..27 masked only vs q 0..511,
    same [128,2048] mask pattern serves both diagonals.
  - softmax denom: DVE-primary / GpSimd (every 4th kv tile) accumulation,
    ones-matmul partition reduce accumulated in PSUM, reciprocal via
    reciprocal_approx_fast, broadcast via gpsimd partition_broadcast.
  - o_proj: PSUM->SBUF copies split Scalar/Vector, bf16 output partials.
"""

import math
import numpy as np
import ml_dtypes

import bass_rust
import concourse.bass as bass
import concourse.mybir as mybir
import concourse.tile as tile
from concourse.vector_clock import ScopedClock
from concourse.bass_utils import run_bass_kernel_spmd

# ---------------------------------------------------------------------------
# Workaround: walrus in this image rejects >1 sem wait on CTRL-class
# instructions (Drain/NoOp). TileContext's tail drain waits on every touched
# logical processor. Split the waits across preceding sync-engine nops.
MAX_WAITS = 1


def _split_waits(nc, inst):
    si = inst.ins.sync_info
    if si is None:
        return
    waits = list(si.on_wait)
    if len(waits) <= MAX_WAITS:
        return
    inst.ins.sync_info = bass_rust.SyncInfo(
        on_wait=waits[:MAX_WAITS], on_update=list(si.on_update)
    )
    rest = waits[MAX_WAITS:]
    while rest:
        extra = nc.sync.nop(nofuse=True)
        extra.ins.sync_info = bass_rust.SyncInfo(on_wait=rest[:MAX_WAITS], on_update=[])
        rest = rest[MAX_WAITS:]


def _drain_and_barrier_split(self, tick_clock, wait_clock):
    nc = self.nc
    carrier = nc.sync.nop(nofuse=True)
    wait_clock.add_sem_waits(carrier.ins, ScopedClock({None: tick_clock.global_clock}))
    _split_waits(nc, carrier)
    nc.sync.drain()
    nc.all_engine_barrier()
    popped = nc._tile_sem_poison_stack.pop()
    assert popped is self._sem_poison
    nc.clear_and_free_semaphores(list(self.sems.allocated().values()))
    nc.all_engine_barrier()


tile.TileContext._drain_and_barrier = _drain_and_barrier_split
# ---------------------------------------------------------------------------

# ---------------------------------------------------------------------------
# General wait-cap legalization: this walrus rejects instructions carrying
# more than a couple of sem waits. Post-process the BIR JSON: hoist overflow
# waits onto engine-matched NoOps inserted immediately before the offender
# (same engine queue -> same ordering semantics).
import json as _json

_CTRL_OPS = {"NoOp", "Drain", "EventSemaphore"}
_CAP_CTRL = 1
_CAP_OTHER = 1
_orig_to_json_bytes = bass.Bass.to_json_bytes


def _legalized_to_json_bytes(self, *a, **k):
    raw = _orig_to_json_bytes(self, *a, **k)
    m = _json.loads(raw)
    ctr = [0]
    changed = False
    for fn in m.get("functions", []):
        for blk in fn.get("blocks", []):
            insts = blk.get("instructions", [])
            out = []
            for ins in insts:
                si = ins.get("sync_info")
                if si:
                    waits = si.get("on_wait") or []
                    cap = _CAP_CTRL if ins.get("opcode") in _CTRL_OPS else _CAP_OTHER
                    if len(waits) > cap:
                        changed = True
                        rest = waits[:-cap]
                        si["on_wait"] = waits[-cap:]
                        while rest:
                            ctr[0] += 1
                            out.append({
                                "debug": ins.get("debug", 0),
                                "engine": ins["engine"],
                                "ins": [], "outs": [],
                                "name": f"{ins['name']}_lw{ctr[0]}",
                                "opcode": "NoOp",
                                "sync_info": {"on_wait": rest[:_CAP_CTRL],
                                              "on_update": []},
                            })
                            rest = rest[_CAP_CTRL:]
                out.append(ins)
            blk["instructions"] = out
    if not changed:
        return raw
    return _json.dumps(m).encode()


bass.Bass.to_json_bytes = _legalized_to_json_bytes
# ---------------------------------------------------------------------------


B, Q, PAST, HID = 1, 1024, 3072, 4096
NH, NKV, HD = 32, 8, 128
KV = PAST + Q           # 4096
NCORES = 8
HPC = NH // NCORES      # 4 query heads per core
ROPE_THETA = 10000.0
EXP_SHIFT = -20.0       # constant softmax shift (cancels exactly per row)

F32 = mybir.dt.float32
BF16 = mybir.dt.bfloat16

N_KT = KV // 128        # 32 kv tiles
N_HK = HID // 128       # 32 hid k-tiles
GRP = 512               # query group width (stage 1)
N_G = Q // GRP          # 2 groups
N_PV = PAST // 128      # 24 past-v tiles
N_SUP = 30              # stage-2 supertiles/head: 28 full-q kv tiles + 2 narrow pairs

LAST_RESULTS = None     # test harness reads exec_time_ns from here


def _build_program():
    nc = bass.Bass()
    hst = nc.declare_dram_parameter("hst", [128, N_HK, Q], BF16, isOutput=False)
    wqt = nc.declare_dram_parameter("wqt", [128, N_HK, HPC * 128], BF16, isOutput=False)
    wkvt = nc.declare_dram_parameter("wkvt", [128, N_HK, 256], BF16, isOutput=False)
    pastkt = nc.declare_dram_parameter("pastkt", [128, PAST], BF16, isOutput=False)
    pastv = nc.declare_dram_parameter("pastv", [128, PAST], BF16, isOutput=False)
    maskt = nc.declare_dram_parameter("maskt", [128, 2048], BF16, isOutput=False)
    # rope tables in [d, seq] layout; q tables pre-scaled by 1/sqrt(HD)
    cosq = nc.declare_dram_parameter("cosq", [128, Q], BF16, isOutput=False)
    sinq = nc.declare_dram_parameter("sinq", [128, Q], BF16, isOutput=False)
    cosk = nc.declare_dram_parameter("cosk", [128, Q], BF16, isOutput=False)
    sink = nc.declare_dram_parameter("sink", [128, Q], BF16, isOutput=False)
    prot = nc.declare_dram_parameter("prot", [128, 128], BF16, isOutput=False)
    sel2 = nc.declare_dram_parameter("sel2", [2, 256], F32, isOutput=False)
    wot = nc.declare_dram_parameter("wot", [128, HPC * HID], BF16, isOutput=False)
    outp = nc.declare_dram_parameter("outp", [Q, HID], BF16, isOutput=True)

    with tile.TileContext(nc) as tc:
        with (
            tc.tile_pool(name="const", bufs=1) as cpool,
            tc.tile_pool(name="kvres", bufs=1) as kvpool,
            tc.tile_pool(name="qt", bufs=1) as qtpool,
            tc.tile_pool(name="attn", bufs=1) as apool,
        ):
            # ones2a/b: lhsT for denominator partition-reduce; row-select into
            # a shared [2, 512] PSUM bank (row 0 = q-half A, row 1 = q-half B)
            ones2a = cpool.tile([128, 2], BF16)
            nc.vector.memset(ones2a[:], 0.0)
            nc.vector.memset(ones2a[:, 0:1], 1.0)
            ones2b = cpool.tile([128, 2], BF16)
            nc.vector.memset(ones2b[:], 0.0)
            nc.vector.memset(ones2b[:, 1:2], 1.0)
            # sel_a/b: lhsT selecting row 0/1 of rc [2, 512] and broadcasting
            # it across all 128 output partitions (DMA'd: partition-sliced
            # memset is rejected by the BIR verifier)
            sel2_sb = cpool.tile([2, 256], F32)
            sel_a = sel2_sb[:, 0:128]
            sel_b = sel2_sb[:, 128:256]
            shift_sb = cpool.tile([128, 1], F32)
            nc.vector.memset(shift_sb[:], EXP_SHIFT)
            prot_sb = cpool.tile([128, 128], BF16)
            mask_sb = cpool.tile([128, 2048], BF16)

            # K_T [128 d, KV] bf16; V packed [128 kv-sub, N_KT*128 d]
            kt_sb = kvpool.tile([128, KV], BF16)
            v_sb = kvpool.tile([128, N_KT * 128], BF16)

            # qT per head [128 d, Q] bf16; attnT per head [128 d, Q] bf16
            qt_sb = [qtpool.tile([128, Q], BF16, tag=f"qt{h}", name=f"qt{h}") for h in range(HPC)]
            at_sb = [apool.tile([128, Q], BF16, tag=f"at{h}", name=f"at{h}") for h in range(HPC)]

            # ---------------- stage 1: QKV projection + RoPE ----------------
            with (
                tc.tile_pool(name="hsw", bufs=1) as hspool,
                tc.tile_pool(name="rope", bufs=2) as rpool,
                tc.tile_pool(name="qkps", bufs=1, space="PSUM") as qkps,
                tc.tile_pool(name="vps", bufs=2, space="PSUM") as vps,
                tc.tile_pool(name="rotps", bufs=1, space="PSUM") as rotps,
            ):
                hs_sb = hspool.tile([128, N_HK, Q], BF16)
                wq_sb = hspool.tile([128, N_HK, HPC * 128], BF16)
                wkv_sb = hspool.tile([128, N_HK, 256], BF16)
                cosq_sb = hspool.tile([128, Q], BF16)
                sinq_sb = hspool.tile([128, Q], BF16)
                cosk_sb = hspool.tile([128, Q], BF16)
                sink_sb = hspool.tile([128, Q], BF16)
                # stage-1-critical loads first, chunked for early start
                for i in range(8):
                    s, e = i * (N_HK // 8), (i + 1) * (N_HK // 8)
                    nc.sync.dma_start(hs_sb[:, s:e, :], hst[:, s:e, :])
                    nc.sync.dma_start(wq_sb[:, s:e, :], wqt[:, s:e, :])
                    if i < 4:
                        s2, e2 = i * (N_HK // 4), (i + 1) * (N_HK // 4)
                        nc.sync.dma_start(wkv_sb[:, s2:e2, :], wkvt[:, s2:e2, :])
                nc.sync.dma_start(cosq_sb[:], cosq[:])
                nc.sync.dma_start(sinq_sb[:], sinq[:])
                nc.sync.dma_start(cosk_sb[:], cosk[:])
                nc.sync.dma_start(sink_sb[:], sink[:])
                nc.sync.dma_start(prot_sb[:], prot[:])
                nc.sync.dma_start(mask_sb[:], maskt[:])
                nc.sync.dma_start(sel2_sb[:], sel2[:])
                nc.sync.dma_start(kt_sb[:, :PAST], pastkt[:])
                nc.sync.dma_start(v_sb[:, : N_PV * 128], pastv[:])

                def rope(dst_bf, src_ps, cos_t, sin_t, g):
                    """dst_bf [128 d, 512 s] <- RoPE applied in [d, s] layout.

                    rot = P_rot.T @ src (PE permutation matmul, sign folded
                    into P_rot); dst = src*cos + rot*sin.
                    """
                    c = cos_t[:, g * GRP:(g + 1) * GRP]
                    s = sin_t[:, g * GRP:(g + 1) * GRP]
                    q_f = rpool.tile([128, GRP], BF16, tag="qf", name="q_f")
                    nc.scalar.copy(q_f[:], src_ps[:])
                    rot_ps = rotps.tile([128, GRP], F32, tag="rot", name="rot_ps")
                    nc.tensor.matmul(rot_ps[:], prot_sb[:], q_f[:],
                                     start=True, stop=True)
                    t1 = rpool.tile([128, GRP], F32, tag="t1", name="t1")
                    nc.vector.tensor_mul(t1[:], src_ps[:], c)
                    t2 = rpool.tile([128, GRP], F32, tag="t2", name="t2")
                    nc.vector.tensor_mul(t2[:], rot_ps[:], s)
                    nc.vector.tensor_add(dst_bf, t1[:], t2[:])

                for g in range(N_G):
                    gsl = slice(g * GRP, (g + 1) * GRP)
                    q_ps = [qkps.tile([128, GRP], F32, tag=f"qps{h}", name=f"qps{h}")
                            for h in range(HPC)]
                    k_ps = qkps.tile([128, GRP], F32, tag="kps", name="k_ps")
                    for k in range(N_HK):
                        rhs = hs_sb[:, k:k + 1, g * GRP:(g + 1) * GRP]
                        for h in range(HPC):
                            nc.tensor.matmul(
                                q_ps[h][:],
                                wq_sb[:, k:k + 1, h * 128:(h + 1) * 128],
                                rhs, start=(k == 0), stop=(k == N_HK - 1),
                            )
                        nc.tensor.matmul(
                            k_ps[:], wkv_sb[:, k:k + 1, 0:128], rhs,
                            start=(k == 0), stop=(k == N_HK - 1),
                        )
                    # v proj: hs-stationary, out [seq, d] per 128-seq tile
                    for st in range(4):
                        gst = g * 4 + st
                        v_ps = vps.tile([128, 128], F32, tag="vp", name="v_ps")
                        for k in range(N_HK):
                            nc.tensor.matmul(
                                v_ps[:],
                                hs_sb[:, k:k + 1, gst * 128:(gst + 1) * 128],
                                wkv_sb[:, k:k + 1, 128:256],
                                start=(k == 0), stop=(k == N_HK - 1),
                            )
                        nc.scalar.copy(
                            v_sb[:, (N_PV + gst) * 128:(N_PV + gst + 1) * 128],
                            v_ps[:],
                        )
                    for h in range(HPC):
                        rope(qt_sb[h][:, gsl], q_ps[h], cosq_sb, sinq_sb, g)
                    rope(kt_sb[:, PAST + g * GRP: PAST + (g + 1) * GRP],
                         k_ps, cosk_sb, sink_sb, g)

            # ------------- stage 2 + 3 (wo loads during stage 2) -------------
            with (
                tc.tile_pool(name="wo", bufs=1) as wopool,
                tc.tile_pool(name="ostage", bufs=2) as ostpool,
            ):
                wo_sb = wopool.tile([128, HPC * HID], BF16)
                for h in range(HPC):
                    nc.sync.dma_start(
                        wo_sb[:, h * HID:(h + 1) * HID],
                        wot[:, h * HID:(h + 1) * HID],
                    )

                # ---------------- stage 2: attention ----------------
                with (
                    tc.tile_pool(name="pt", bufs=3) as ptpool,
                    tc.tile_pool(name="softm", bufs=2) as smpool,
                    tc.tile_pool(name="scps", bufs=2, space="PSUM") as scps,
                    tc.tile_pool(name="aps", bufs=1, space="PSUM") as aps,
                    tc.tile_pool(name="dps", bufs=1, space="PSUM") as dps,
                ):
                    for h in range(HPC):
                        a_ps = aps.tile([128, Q], F32, tag="aacc", name="a_ps")
                        ds_ps = dps.tile([2, GRP], F32, tag="dsum", name="ds_ps")
                        # denominator partials on DVE in bf16 (2 accumulators
                        # -> rounding error ~sqrt(15)*2^-9, well within budget)
                        dn0 = smpool.tile([128, Q], BF16, tag="dn0", name="dn0")
                        dn1 = smpool.tile([128, Q], BF16, tag="dn1", name="dn1")

                        def emit_attn(prev):
                            """Attn accumulation for the previous supertile
                            (lagged so the PE never waits on this supertile's
                            exp)."""
                            _, pt, plan = prev
                            for (kt, col, qoff) in plan:
                                nc.tensor.matmul(
                                    a_ps[:, qoff:qoff + GRP],
                                    v_sb[:, kt * 128:(kt + 1) * 128],
                                    pt[:, col:col + GRP],
                                    start=(kt == 0),
                                    stop=(kt == 27 if qoff == 0 else kt == 31),
                                )

                        def dn_accum(jj, pt, plan):
                            if jj < 28:
                                dn = dn0 if jj % 2 == 0 else dn1
                                if jj < 2:
                                    nc.vector.tensor_copy(dn[:], pt[:])
                                else:
                                    nc.vector.tensor_add(dn[:], dn[:], pt[:])
                            else:
                                for (kt, col, qoff) in plan:
                                    dn = dn0 if kt % 2 == 0 else dn1
                                    nc.vector.tensor_add(
                                        dn[:, qoff:qoff + GRP],
                                        dn[:, qoff:qoff + GRP],
                                        pt[:, col:col + GRP],
                                    )

                        prev = None
                        for jj in range(N_SUP):
                            s_sup = scps.tile([128, 1024], F32, tag="ss", name="s_sup")
                            pt = ptpool.tile([128, 1024], BF16, tag="pt", name="pt")
                            plan = []
                            if jj < 28:
                                # kv tile jj x full q
                                kt = jj
                                for ii in range(2):
                                    nc.tensor.matmul(
                                        s_sup[:, ii * GRP:(ii + 1) * GRP],
                                        kt_sb[:, kt * 128:(kt + 1) * 128],
                                        qt_sb[h][:, ii * GRP:(ii + 1) * GRP],
                                        start=True, stop=True,
                                    )
                                    plan.append((kt, ii * GRP, ii * GRP))
                                if jj >= 24:  # diagonal: mask vs q 0..511
                                    nc.vector.tensor_add(
                                        s_sup[:, 0:GRP], s_sup[:, 0:GRP],
                                        mask_sb[:, (kt - 24) * GRP:(kt - 23) * GRP],
                                    )
                            else:
                                # narrow pair: kv (28,29) or (30,31) x q-half B
                                for c in range(2):
                                    kt = 28 + 2 * (jj - 28) + c
                                    nc.tensor.matmul(
                                        s_sup[:, c * GRP:(c + 1) * GRP],
                                        kt_sb[:, kt * 128:(kt + 1) * 128],
                                        qt_sb[h][:, GRP:Q],
                                        start=True, stop=True,
                                    )
                                    plan.append((kt, c * GRP, GRP))
                                    nc.vector.tensor_add(
                                        s_sup[:, c * GRP:(c + 1) * GRP],
                                        s_sup[:, c * GRP:(c + 1) * GRP],
                                        mask_sb[:, (kt - 28) * GRP:(kt - 27) * GRP],
                                    )
                            nc.scalar.activation(
                                pt[:], s_sup[:],
                                mybir.ActivationFunctionType.Exp,
                                bias=shift_sb[:], scale=1.0,
                            )
                            dn_accum(jj, pt, plan)
                            if prev is not None:
                                emit_attn(prev)
                            prev = (jj, pt, plan)
                        emit_attn(prev)
                        # partition-reduce the two dn accumulators: row 0 of
                        # ds_ps = q-half A denom, row 1 = q-half B denom
                        for idx, (sel, dn, hoff) in enumerate(
                            [(ones2a, dn0, 0), (ones2a, dn1, 0),
                             (ones2b, dn0, GRP), (ones2b, dn1, GRP)]
                        ):
                            nc.tensor.matmul(
                                ds_ps[:], sel[:], dn[:, hoff:hoff + GRP],
                                start=(idx == 0), stop=(idx == 3),
                            )

                        # copy a_ps out unnormalized right away (frees the
                        # a_ps/ds_ps banks for the next head); the recip ->
                        # broadcast -> normalize chain runs off-critical-path
                        au_sb = smpool.tile([128, Q], BF16, tag="atu", name="au_sb")
                        nc.vector.tensor_copy(au_sb[:], a_ps[:])
                        rc_sb = smpool.tile([2, GRP], F32, tag="recip", name="rc_sb")
                        bc_sb = smpool.tile([128, Q], F32, tag="bcast", name="bc_sb")
                        nc.vector.reciprocal(rc_sb[:], ds_ps[:])
                        for half in range(2):
                            hsl = slice(half * GRP, (half + 1) * GRP)
                            bc_ps = dps.tile([128, GRP], F32, tag="bcps", name="bc_ps")
                            nc.tensor.matmul(bc_ps[:],
                                             sel_a if half == 0 else sel_b,
                                             rc_sb[:], start=True, stop=True)
                            nc.scalar.copy(bc_sb[:, hsl], bc_ps[:])
                        nc.vector.tensor_mul(at_sb[h][:], au_sb[:], bc_sb[:])

                # ---------------- stage 3: o_proj partial ----------------
                with tc.tile_pool(name="ops", bufs=2, space="PSUM") as opps:
                    for st in range(8):
                        for half in range(2):
                            o_sb = ostpool.tile([128, 2048], BF16, tag="osb",
                                                name="o_sb")
                            o_ps = opps.tile([128, 2048], F32, tag="ops",
                                             name="o_ps")
                            for h in range(HPC):
                                for nn in range(4):
                                    n = half * 4 + nn
                                    nc.tensor.matmul(
                                        o_ps[:, nn * 512:(nn + 1) * 512],
                                        at_sb[h][:, st * 128:(st + 1) * 128],
                                        wo_sb[:, h * HID + n * 512:
                                              h * HID + (n + 1) * 512],
                                        start=(h == 0), stop=(h == HPC - 1),
                                    )
                            if (st + half) % 2 == 0:
                                nc.scalar.copy(o_sb[:], o_ps[:])
                            else:
                                nc.vector.tensor_copy(o_sb[:], o_ps[:])
                            nc.sync.dma_start(
                                outp[st * 128:(st + 1) * 128,
                                     half * 2048:(half + 1) * 2048],
                                o_sb[:],
                            )
    return nc


def _pack_ktiles(a, tile_rows=128):
    """[R, C] -> [128, (R//128)*C] with k-tile kt at cols [kt*C:(kt+1)*C]."""
    r, c = a.shape
    n = r // tile_rows
    return np.ascontiguousarray(
        a.reshape(n, tile_rows, c).transpose(1, 0, 2).reshape(tile_rows, n * c)
    )


def _rope_tables_ds(position_ids):
    """cos/sin in [d, s] layout: [128, Q] f64."""
    pos = np.asarray(position_ids).reshape(-1).astype(np.float64)      # [Q]
    inv_freq = 1.0 / (ROPE_THETA ** (np.arange(0, HD, 2, dtype=np.float64) / HD))
    ang_half = np.outer(inv_freq, pos)                                 # [64, Q]
    ang = np.concatenate([ang_half, ang_half], axis=0)                 # [128, Q]
    return np.cos(ang), np.sin(ang)


def kernel(hidden_states, attention_mask, position_ids, past_k, past_v,
           Wq, Wk, Wv, Wo):
    global LAST_RESULTS
    bf = ml_dtypes.bfloat16

    hs = np.asarray(hidden_states, np.float32).reshape(Q, HID)
    mask = np.asarray(attention_mask, np.float32).reshape(Q, KV)
    cos_d, sin_d = _rope_tables_ds(position_ids)

    scale = 1.0 / math.sqrt(HD)
    cosq_t = (cos_d * scale).astype(bf)
    sinq_t = (sin_d * scale).astype(bf)
    cosk_t = cos_d.astype(bf)
    sink_t = sin_d.astype(bf)

    # rotate-half permutation with sign: rot[d] = -x[d+64] (d<64); x[d-64]
    prot_np = np.zeros((128, 128), np.float32)
    for dd in range(64):
        prot_np[dd + 64, dd] = -1.0     # lhsT[d', d]: rot[d] += P[d', d] * x[d']
        prot_np[dd, dd + 64] = 1.0
    prot_t = prot_np.astype(bf)

    # diagonal masks: [128 kv, 4 tiles * 512 q]: kv tile 24+m vs queries
    # 0..511 (identical pattern to kv tile 28+m vs queries 512..1023)
    mask_t = np.empty((128, 2048), np.float32)
    for m in range(4):
        kt = 24 + m
        mask_t[:, m * 512:(m + 1) * 512] = mask[0:512, kt * 128:(kt + 1) * 128].T
    mask_t = mask_t.astype(bf)

    sel2_np = np.zeros((2, 256), np.float32)
    sel2_np[0, 0:128] = 1.0      # sel_a: broadcast rc row 0
    sel2_np[1, 128:256] = 1.0    # sel_b: broadcast rc row 1

    hst = _pack_ktiles(np.ascontiguousarray(hs.T)).astype(bf)      # [128, 32, 1024]
    hst = hst.reshape(128, N_HK, Q)

    nc = _build_program()
    in_maps = []
    for c in range(NCORES):
        qs = slice(c * HPC * HD, (c + 1) * HPC * HD)
        ks = slice(c * HD, (c + 1) * HD)
        wq_c = _pack_ktiles(
            np.ascontiguousarray(Wq[qs, :].T)
        ).astype(bf).reshape(128, N_HK, HPC * 128)
        wk_c = np.ascontiguousarray(Wk[ks, :].T)                   # [4096, 128]
        wv_c = np.ascontiguousarray(Wv[ks, :].T)
        wkv_c = _pack_ktiles(
            np.concatenate([wk_c, wv_c], axis=1)
        ).astype(bf).reshape(128, N_HK, 256)
        pkt = np.ascontiguousarray(past_k[0, c].T).astype(bf)      # [128, 3072]
        pv = _pack_ktiles(np.ascontiguousarray(past_v[0, c])).astype(bf)
        wo_c = _pack_ktiles(
            np.ascontiguousarray(Wo[:, qs].T)).astype(bf)          # [128, 4*4096]
        in_maps.append({
            "hst": hst, "wqt": wq_c, "wkvt": wkv_c, "pastkt": pkt,
            "pastv": pv, "maskt": mask_t, "cosq": cosq_t, "sinq": sinq_t,
            "cosk": cosk_t, "sink": sink_t, "prot": prot_t, "sel2": sel2_np,
            "wot": wo_c,
        })

    res = run_bass_kernel_spmd(nc, in_maps, list(range(NCORES)))
    LAST_RESULTS = res
    out = np.zeros((Q, HID), np.float32)
    for c in range(NCORES):
        out += np.asarray(res.results[c]["outp"], dtype=np.float32)
    return out.reshape(B, Q, HID)


# revision 3
# speedup vs baseline: 1.0172x; 1.0172x over previous
"""Llama GQA attention (B=1, Q=1024, PAST=3072, HID=4096, NH=32, NKV=8, HD=128)
tensor-parallel over heads across 8 NeuronCores.

Per core c: kv head c, query heads 4c..4c+3. Each core computes its partial
o_proj contribution [1024, 4096] in bf16; the host sums the 8 partials in f32.

v2 layout (vs v1): engineered to unload the DVE (v1 bottleneck: 441us busy).
  - q/k proj W-stationary: out is [d, seq] (born transposed, no PE transposes).
    RoPE rotate-half via a PE permutation matmul; combine = 3 DVE TT ops.
  - v proj hs-stationary: out is [seq, d] directly in attn lhsT layout.
  - scores land in bf16 PSUM supertiles [128, 2048] (2 kv tiles x 1024 q);
    exp is ONE fused ACTIVATE per supertile reading PSUM directly. No mask
    add except on the diagonal tiles (DVE, in-place in PSUM).
  - causal skip: kv tiles 28..31 only computed against queries 512..1023
    (one extra narrow-quad supertile); kv 24..27 masked only vs q 0..511,
    same [128,2048] mask pattern serves both diagonals.
  - softmax denom: DVE-primary / GpSimd (every 4th kv tile) accumulation,
    ones-matmul partition reduce accumulated in PSUM, reciprocal via
    reciprocal_approx_fast, broadcast via gpsimd partition_broadcast.
  - o_proj: PSUM->SBUF copies split Scalar/Vector, bf16 output partials.
"""

import math
import numpy as np
import ml_dtypes

import bass_rust
import concourse.bass as bass
import concourse.mybir as mybir
import concourse.tile as tile
from concourse.vector_clock import ScopedClock
from concourse.bass_utils import run_bass_kernel_spmd

# ---------------------------------------------------------------------------
# Workaround: walrus in this image rejects >1 sem wait on CTRL-class
# instructions (Drain/NoOp). TileContext's tail drain waits on every touched
# logical processor. Split the waits across preceding sync-engine nops.
MAX_WAITS = 1


def _split_waits(nc, inst):
    si = inst.ins.sync_info
    if si is None:
        return
    waits = list(si.on_wait)
    if len(waits) <= MAX_WAITS:
        return
    inst.ins.sync_info = bass_rust.SyncInfo(
        on_wait=waits[:MAX_WAITS], on_update=list(si.on_update)
    )
    rest = waits[MAX_WAITS:]
    while rest:
        extra = nc.sync.nop(nofuse=True)
        extra.ins.sync_info = bass_rust.SyncInfo(on_wait=rest[:MAX_WAITS], on_update=[])
        rest = rest[MAX_WAITS:]


def _drain_and_barrier_split(self, tick_clock, wait_clock):
    nc = self.nc
    carrier = nc.sync.nop(nofuse=True)
    wait_clock.add_sem_waits(carrier.ins, ScopedClock({None: tick_clock.global_clock}))
    _split_waits(nc, carrier)
    nc.sync.drain()
    nc.all_engine_barrier()
    popped = nc._tile_sem_poison_stack.pop()
    assert popped is self._sem_poison
    nc.clear_and_free_semaphores(list(self.sems.allocated().values()))
    nc.all_engine_barrier()


tile.TileContext._drain_and_barrier = _drain_and_barrier_split
# ---------------------------------------------------------------------------

# ---------------------------------------------------------------------------
# General wait-cap legalization: this walrus rejects instructions carrying
# more than a couple of sem waits. Post-process the BIR JSON: hoist overflow
# waits onto engine-matched NoOps inserted immediately before the offender
# (same engine queue -> same ordering semantics).
import json as _json

_CTRL_OPS = {"NoOp", "Drain", "EventSemaphore"}
_CAP_CTRL = 1
_CAP_OTHER = 1
_orig_to_json_bytes = bass.Bass.to_json_bytes


def _legalized_to_json_bytes(self, *a, **k):
    raw = _orig_to_json_bytes(self, *a, **k)
    m = _json.loads(raw)
    ctr = [0]
    changed = False
    for fn in m.get("functions", []):
        for blk in fn.get("blocks", []):
            insts = blk.get("instructions", [])
            out = []
            for ins in insts:
                si = ins.get("sync_info")
                if si:
                    waits = si.get("on_wait") or []
                    cap = _CAP_CTRL if ins.get("opcode") in _CTRL_OPS else _CAP_OTHER
                    if len(waits) > cap:
                        changed = True
                        rest = waits[:-cap]
                        si["on_wait"] = waits[-cap:]
                        while rest:
                            ctr[0] += 1
                            out.append({
                                "debug": ins.get("debug", 0),
                                "engine": ins["engine"],
                                "ins": [], "outs": [],
                                "name": f"{ins['name']}_lw{ctr[0]}",
                                "opcode": "NoOp",
                                "sync_info": {"on_wait": rest[:_CAP_CTRL],
                                              "on_update": []},
                            })
                            rest = rest[_CAP_CTRL:]
                out.append(ins)
            blk["instructions"] = out
    if not changed:
        return raw
    return _json.dumps(m).encode()


bass.Bass.to_json_bytes = _legalized_to_json_bytes
# ---------------------------------------------------------------------------


B, Q, PAST, HID = 1, 1024, 3072, 4096
NH, NKV, HD = 32, 8, 128
KV = PAST + Q           # 4096
NCORES = 8
HPC = NH // NCORES      # 4 query heads per core
ROPE_THETA = 10000.0
EXP_SHIFT = -20.0       # constant softmax shift (cancels exactly per row)

F32 = mybir.dt.float32
BF16 = mybir.dt.bfloat16

N_KT = KV // 128        # 32 kv tiles
N_HK = HID // 128       # 32 hid k-tiles
GRP = 512               # query group width (stage 1)
N_G = Q // GRP          # 2 groups
N_PV = PAST // 128      # 24 past-v tiles
N_SUP = 30              # stage-2 supertiles/head: 28 full-q kv tiles + 2 narrow pairs

LAST_RESULTS = None     # test harness reads exec_time_ns from here


def _build_program():
    nc = bass.Bass()
    hst = nc.declare_dram_parameter("hst", [128, N_HK, Q], BF16, isOutput=False)
    wqt = nc.declare_dram_parameter("wqt", [128, N_HK, HPC * 128], BF16, isOutput=False)
    wkvt = nc.declare_dram_parameter("wkvt", [128, N_HK, 256], BF16, isOutput=False)
    pastkt = nc.declare_dram_parameter("pastkt", [128, PAST], BF16, isOutput=False)
    pastv = nc.declare_dram_parameter("pastv", [128, PAST], BF16, isOutput=False)
    maskt = nc.declare_dram_parameter("maskt", [128, 2048], BF16, isOutput=False)
    # rope tables in [d, seq] layout; q tables pre-scaled by 1/sqrt(HD)
    cosq = nc.declare_dram_parameter("cosq", [128, Q], BF16, isOutput=False)
    sinq = nc.declare_dram_parameter("sinq", [128, Q], BF16, isOutput=False)
    cosk = nc.declare_dram_parameter("cosk", [128, Q], BF16, isOutput=False)
    sink = nc.declare_dram_parameter("sink", [128, Q], BF16, isOutput=False)
    prot = nc.declare_dram_parameter("prot", [128, 128], BF16, isOutput=False)
    sel2 = nc.declare_dram_parameter("sel2", [2, 256], F32, isOutput=False)
    wot = nc.declare_dram_parameter("wot", [128, HPC * HID], BF16, isOutput=False)
    outp = nc.declare_dram_parameter("outp", [Q, HID], BF16, isOutput=True)

    with tile.TileContext(nc) as tc:
        with (
            tc.tile_pool(name="const", bufs=1) as cpool,
            tc.tile_pool(name="kvres", bufs=1) as kvpool,
            tc.tile_pool(name="qt", bufs=1) as qtpool,
            tc.tile_pool(name="attn", bufs=1) as apool,
        ):
            # ones2a/b: lhsT for denominator partition-reduce; row-select into
            # a shared [2, 512] PSUM bank (row 0 = q-half A, row 1 = q-half B)
            ones2a = cpool.tile([128, 2], BF16)
            nc.vector.memset(ones2a[:], 0.0)
            nc.vector.memset(ones2a[:, 0:1], 1.0)
            ones2b = cpool.tile([128, 2], BF16)
            nc.vector.memset(ones2b[:], 0.0)
            nc.vector.memset(ones2b[:, 1:2], 1.0)
            # sel_a/b: lhsT selecting row 0/1 of rc [2, 512] and broadcasting
            # it across all 128 output partitions (DMA'd: partition-sliced
            # memset is rejected by the BIR verifier)
            sel2_sb = cpool.tile([2, 256], F32)
            sel_a = sel2_sb[:, 0:128]
            sel_b = sel2_sb[:, 128:256]
            shift_sb = cpool.tile([128, 1], F32)
            nc.vector.memset(shift_sb[:], EXP_SHIFT)
            prot_sb = cpool.tile([128, 128], BF16)
            mask_sb = cpool.tile([128, 2048], BF16)

            # K_T [128 d, KV] bf16; V packed [128 kv-sub, N_KT*128 d]
            kt_sb = kvpool.tile([128, KV], BF16)
            v_sb = kvpool.tile([128, N_KT * 128], BF16)

            # qT per head [128 d, Q] bf16; attnT per head [128 d, Q] bf16
            qt_sb = [qtpool.tile([128, Q], BF16, tag=f"qt{h}", name=f"qt{h}") for h in range(HPC)]
            at_sb = [apool.tile([128, Q], BF16, tag=f"at{h}", name=f"at{h}") for h in range(HPC)]

            # ---------------- stage 1: QKV projection + RoPE ----------------
            with (
                tc.tile_pool(name="hsw", bufs=1) as hspool,
                tc.tile_pool(name="rope", bufs=2) as rpool,
                tc.tile_pool(name="qkps", bufs=1, space="PSUM") as qkps,
                tc.tile_pool(name="vps", bufs=2, space="PSUM") as vps,
                tc.tile_pool(name="rotps", bufs=1, space="PSUM") as rotps,
            ):
                hs_sb = hspool.tile([128, N_HK, Q], BF16)
                wq_sb = hspool.tile([128, N_HK, HPC * 128], BF16)
                wkv_sb = hspool.tile([128, N_HK, 256], BF16)
                cosq_sb = hspool.tile([128, Q], BF16)
                sinq_sb = hspool.tile([128, Q], BF16)
                cosk_sb = hspool.tile([128, Q], BF16)
                sink_sb = hspool.tile([128, Q], BF16)
                # stage-1-critical loads first, chunked for early start
                for i in range(8):
                    s, e = i * (N_HK // 8), (i + 1) * (N_HK // 8)
                    nc.sync.dma_start(hs_sb[:, s:e, :], hst[:, s:e, :])
                    nc.sync.dma_start(wq_sb[:, s:e, :], wqt[:, s:e, :])
                    if i < 4:
                        s2, e2 = i * (N_HK // 4), (i + 1) * (N_HK // 4)
                        nc.sync.dma_start(wkv_sb[:, s2:e2, :], wkvt[:, s2:e2, :])
                nc.sync.dma_start(cosq_sb[:], cosq[:])
                nc.sync.dma_start(sinq_sb[:], sinq[:])
                nc.sync.dma_start(cosk_sb[:], cosk[:])
                nc.sync.dma_start(sink_sb[:], sink[:])
                nc.sync.dma_start(prot_sb[:], prot[:])
                nc.sync.dma_start(mask_sb[:], maskt[:])
                nc.sync.dma_start(sel2_sb[:], sel2[:])
                nc.sync.dma_start(kt_sb[:, :PAST], pastkt[:])
                nc.sync.dma_start(v_sb[:, : N_PV * 128], pastv[:])

                def rope(dst_bf, src_ps, cos_t, sin_t, g):
                    """dst_bf [128 d, 512 s] <- RoPE applied in [d, s] layout.

                    rot = P_rot.T @ src (PE permutation matmul, sign folded
                    into P_rot); dst = src*cos + rot*sin.
                    """
                    c = cos_t[:, g * GRP:(g + 1) * GRP]
                    s = sin_t[:, g * GRP:(g + 1) * GRP]
                    q_f = rpool.tile([128, GRP], BF16, tag="qf", name="q_f")
                    nc.scalar.copy(q_f[:], src_ps[:])
                    rot_ps = rotps.tile([128, GRP], F32, tag="rot", name="rot_ps")
                    nc.tensor.matmul(rot_ps[:], prot_sb[:], q_f[:],
                                     start=True, stop=True)
                    t1 = rpool.tile([128, GRP], F32, tag="t1", name="t1")
                    nc.vector.tensor_mul(t1[:], src_ps[:], c)
                    t2 = rpool.tile([128, GRP], F32, tag="t2", name="t2")
                    nc.vector.tensor_mul(t2[:], rot_ps[:], s)
                    nc.vector.tensor_add(dst_bf, t1[:], t2[:])

                for g in range(N_G):
                    gsl = slice(g * GRP, (g + 1) * GRP)
                    q_ps = [qkps.tile([128, GRP], F32, tag=f"qps{h}", name=f"qps{h}")
                            for h in range(HPC)]
                    k_ps = qkps.tile([128, GRP], F32, tag="kps", name="k_ps")
                    for k in range(N_HK):
                        rhs = hs_sb[:, k:k + 1, g * GRP:(g + 1) * GRP]
                        for h in range(HPC):
                            nc.tensor.matmul(
                                q_ps[h][:],
                                wq_sb[:, k:k + 1, h * 128:(h + 1) * 128],
                                rhs, start=(k == 0), stop=(k == N_HK - 1),
                            )
                        nc.tensor.matmul(
                            k_ps[:], wkv_sb[:, k:k + 1, 0:128], rhs,
                            start=(k == 0), stop=(k == N_HK - 1),
                        )
                    # v proj: hs-stationary, out [seq, d] per 128-seq tile
                    for st in range(4):
                        gst = g * 4 + st
                        v_ps = vps.tile([128, 128], F32, tag="vp", name="v_ps")
                        for k in range(N_HK):
                            nc.tensor.matmul(
                                v_ps[:],
                                hs_sb[:, k:k + 1, gst * 128:(gst + 1) * 128],
                                wkv_sb[:, k:k + 1, 128:256],
                                start=(k == 0), stop=(k == N_HK - 1),
                            )
                        nc.scalar.copy(
                            v_sb[:, (N_PV + gst) * 128:(N_PV + gst + 1) * 128],
                            v_ps[:],
                        )
                    for h in range(HPC):
                        rope(qt_sb[h][:, gsl], q_ps[h], cosq_sb, sinq_sb, g)
                    rope(kt_sb[:, PAST + g * GRP: PAST + (g + 1) * GRP],
                         k_ps, cosk_sb, sink_sb, g)

            # ------------- stage 2 + 3 (wo loads during stage 2) -------------
            with (
                tc.tile_pool(name="wo", bufs=1) as wopool,
                tc.tile_pool(name="ostage", bufs=2) as ostpool,
            ):
                wo_sb = wopool.tile([128, HPC * HID], BF16)
                for h in range(HPC):
                    nc.sync.dma_start(
                        wo_sb[:, h * HID:(h + 1) * HID],
                        wot[:, h * HID:(h + 1) * HID],
                    )

                # ---------------- stage 2: attention ----------------
                with (
                    tc.tile_pool(name="pt", bufs=6) as ptpool,
                    tc.tile_pool(name="softm", bufs=2) as smpool,
                    tc.tile_pool(name="scps", bufs=2, space="PSUM") as scps,
                    tc.tile_pool(name="aps", bufs=1, space="PSUM") as aps,
                    tc.tile_pool(name="dps", bufs=1, space="PSUM") as dps,
                ):
                    for h in range(HPC):
                        a_ps = aps.tile([128, Q], F32, tag="aacc", name="a_ps")
                        ds_ps = dps.tile([2, GRP], F32, tag="dsum", name="ds_ps")
                        # denominator partials on DVE in bf16 (2 accumulators
                        # -> rounding error ~sqrt(15)*2^-9, well within budget)
                        dn0 = smpool.tile([128, Q], BF16, tag="dn0", name="dn0")
                        dn1 = smpool.tile([128, Q], BF16, tag="dn1", name="dn1")

                        def emit_attn(prev):
                            """Attn accumulation for the previous supertile
                            (lagged so the PE never waits on this supertile's
                            exp)."""
                            _, pt, plan = prev
                            for (kt, col, qoff) in plan:
                                nc.tensor.matmul(
                                    a_ps[:, qoff:qoff + GRP],
                                    v_sb[:, kt * 128:(kt + 1) * 128],
                                    pt[:, col:col + GRP],
                                    start=(kt == 0),
                                    stop=(kt == 27 if qoff == 0 else kt == 31),
                                )

                        def dn_accum(jj, pt, plan):
                            if jj < 28:
                                dn = dn0 if jj % 2 == 0 else dn1
                                if jj < 2:
                                    nc.vector.tensor_copy(dn[:], pt[:])
                                else:
                                    nc.vector.tensor_add(dn[:], dn[:], pt[:])
                            else:
                                for (kt, col, qoff) in plan:
                                    dn = dn0 if kt % 2 == 0 else dn1
                                    nc.vector.tensor_add(
                                        dn[:, qoff:qoff + GRP],
                                        dn[:, qoff:qoff + GRP],
                                        pt[:, col:col + GRP],
                                    )

                        pend = []
                        for jj in range(N_SUP):
                            s_sup = scps.tile([128, 1024], F32, tag="ss", name="s_sup")
                            pt = ptpool.tile([128, 1024], BF16, tag="pt", name="pt")
                            plan = []
                            if jj < 28:
                                # kv tile jj x full q
                                kt = jj
                                for ii in range(2):
                                    nc.tensor.matmul(
                                        s_sup[:, ii * GRP:(ii + 1) * GRP],
                                        kt_sb[:, kt * 128:(kt + 1) * 128],
                                        qt_sb[h][:, ii * GRP:(ii + 1) * GRP],
                                        start=True, stop=True,
                                    )
                                    plan.append((kt, ii * GRP, ii * GRP))
                            else:
                                # narrow pair: kv (28,29) or (30,31) x q-half B
                                for c in range(2):
                                    kt = 28 + 2 * (jj - 28) + c
                                    nc.tensor.matmul(
                                        s_sup[:, c * GRP:(c + 1) * GRP],
                                        kt_sb[:, kt * 128:(kt + 1) * 128],
                                        qt_sb[h][:, GRP:Q],
                                        start=True, stop=True,
                                    )
                                    plan.append((kt, c * GRP, GRP))
                            nc.scalar.activation(
                                pt[:], s_sup[:],
                                mybir.ActivationFunctionType.Exp,
                                bias=shift_sb[:], scale=1.0,
                            )
                            if jj >= 24:
                                if jj < 28:
                                    nc.gpsimd.tensor_mul(
                                        pt[:, 0:GRP], pt[:, 0:GRP],
                                        mask_sb[:, (jj - 24) * GRP:(jj - 23) * GRP],
                                    )
                                else:
                                    for (kt, col, qoff) in plan:
                                        nc.gpsimd.tensor_mul(
                                            pt[:, col:col + GRP],
                                            pt[:, col:col + GRP],
                                            mask_sb[:, (kt - 28) * GRP:(kt - 27) * GRP],
                                        )
                            dn_accum(jj, pt, plan)
                            pend.append((jj, pt, plan))
                            if len(pend) > 2:
                                emit_attn(pend.pop(0))
                        for ent in pend:
                            emit_attn(ent)
                        # partition-reduce the two dn accumulators: row 0 of
                        # ds_ps = q-half A denom, row 1 = q-half B denom
                        for idx, (sel, dn, hoff) in enumerate(
                            [(ones2a, dn0, 0), (ones2a, dn1, 0),
                             (ones2b, dn0, GRP), (ones2b, dn1, GRP)]
                        ):
                            nc.tensor.matmul(
                                ds_ps[:], sel[:], dn[:, hoff:hoff + GRP],
                                start=(idx == 0), stop=(idx == 3),
                            )

                        # copy a_ps out unnormalized right away (frees the
                        # a_ps/ds_ps banks for the next head); the recip ->
                        # broadcast -> normalize chain runs off-critical-path
                        au_sb = smpool.tile([128, Q], BF16, tag="atu", name="au_sb")
                        nc.vector.tensor_copy(au_sb[:], a_ps[:])
                        rc_sb = smpool.tile([2, GRP], F32, tag="recip", name="rc_sb")
                        bc_sb = smpool.tile([128, Q], F32, tag="bcast", name="bc_sb")
                        nc.vector.reciprocal(rc_sb[:], ds_ps[:])
                        for half in range(2):
                            hsl = slice(half * GRP, (half + 1) * GRP)
                            bc_ps = dps.tile([128, GRP], F32, tag="bcps", name="bc_ps")
                            nc.tensor.matmul(bc_ps[:],
                                             sel_a if half == 0 else sel_b,
                                             rc_sb[:], start=True, stop=True)
                            nc.vector.tensor_copy(bc_sb[:, hsl], bc_ps[:])
                        nc.vector.tensor_mul(at_sb[h][:], au_sb[:], bc_sb[:])

                # ---------------- stage 3: o_proj partial ----------------
                with tc.tile_pool(name="ops", bufs=2, space="PSUM") as opps:
                    for st in range(8):
                        for half in range(2):
                            o_sb = ostpool.tile([128, 2048], BF16, tag="osb",
                                                name="o_sb")
                            o_ps = opps.tile([128, 2048], F32, tag="ops",
                                             name="o_ps")
                            for h in range(HPC):
                                for nn in range(4):
                                    n = half * 4 + nn
                                    nc.tensor.matmul(
                                        o_ps[:, nn * 512:(nn + 1) * 512],
                                        at_sb[h][:, st * 128:(st + 1) * 128],
                                        wo_sb[:, h * HID + n * 512:
                                              h * HID + (n + 1) * 512],
                                        start=(h == 0), stop=(h == HPC - 1),
                                    )
                            if (st + half) % 2 == 0:
                                nc.scalar.copy(o_sb[:], o_ps[:])
                            else:
                                nc.vector.tensor_copy(o_sb[:], o_ps[:])
                            nc.sync.dma_start(
                                outp[st * 128:(st + 1) * 128,
                                     half * 2048:(half + 1) * 2048],
                                o_sb[:],
                            )
    return nc


def _pack_ktiles(a, tile_rows=128):
    """[R, C] -> [128, (R//128)*C] with k-tile kt at cols [kt*C:(kt+1)*C]."""
    r, c = a.shape
    n = r // tile_rows
    return np.ascontiguousarray(
        a.reshape(n, tile_rows, c).transpose(1, 0, 2).reshape(tile_rows, n * c)
    )


def _rope_tables_ds(position_ids):
    """cos/sin in [d, s] layout: [128, Q] f64."""
    pos = np.asarray(position_ids).reshape(-1).astype(np.float64)      # [Q]
    inv_freq = 1.0 / (ROPE_THETA ** (np.arange(0, HD, 2, dtype=np.float64) / HD))
    ang_half = np.outer(inv_freq, pos)                                 # [64, Q]
    ang = np.concatenate([ang_half, ang_half], axis=0)                 # [128, Q]
    return np.cos(ang), np.sin(ang)


def kernel(hidden_states, attention_mask, position_ids, past_k, past_v,
           Wq, Wk, Wv, Wo):
    global LAST_RESULTS
    bf = ml_dtypes.bfloat16

    hs = np.asarray(hidden_states, np.float32).reshape(Q, HID)
    mask = np.asarray(attention_mask, np.float32).reshape(Q, KV)
    cos_d, sin_d = _rope_tables_ds(position_ids)

    scale = 1.0 / math.sqrt(HD)
    cosq_t = (cos_d * scale).astype(bf)
    sinq_t = (sin_d * scale).astype(bf)
    cosk_t = cos_d.astype(bf)
    sink_t = sin_d.astype(bf)

    # rotate-half permutation with sign: rot[d] = -x[d+64] (d<64); x[d-64]
    prot_np = np.zeros((128, 128), np.float32)
    for dd in range(64):
        prot_np[dd + 64, dd] = -1.0     # lhsT[d', d]: rot[d] += P[d', d] * x[d']
        prot_np[dd, dd + 64] = 1.0
    prot_t = prot_np.astype(bf)

    # diagonal masks: [128 kv, 4 tiles * 512 q]: kv tile 24+m vs queries
    # 0..511 (identical pattern to kv tile 28+m vs queries 512..1023)
    mask_t = np.empty((128, 2048), np.float32)
    for m in range(4):
        kt = 24 + m
        blk = mask[0:512, kt * 128:(kt + 1) * 128].T
        mask_t[:, m * 512:(m + 1) * 512] = (blk == 0.0).astype(np.float32)
    mask_t = mask_t.astype(bf)

    sel2_np = np.zeros((2, 256), np.float32)
    sel2_np[0, 0:128] = 1.0      # sel_a: broadcast rc row 0
    sel2_np[1, 128:256] = 1.0    # sel_b: broadcast rc row 1

    hst = _pack_ktiles(np.ascontiguousarray(hs.T)).astype(bf)      # [128, 32, 1024]
    hst = hst.reshape(128, N_HK, Q)

    nc = _build_program()
    in_maps = []
    for c in range(NCORES):
        qs = slice(c * HPC * HD, (c + 1) * HPC * HD)
        ks = slice(c * HD, (c + 1) * HD)
        wq_c = _pack_ktiles(
            np.ascontiguousarray(Wq[qs, :].T)
        ).astype(bf).reshape(128, N_HK, HPC * 128)
        wk_c = np.ascontiguousarray(Wk[ks, :].T)                   # [4096, 128]
        wv_c = np.ascontiguousarray(Wv[ks, :].T)
        wkv_c = _pack_ktiles(
            np.concatenate([wk_c, wv_c], axis=1)
        ).astype(bf).reshape(128, N_HK, 256)
        pkt = np.ascontiguousarray(past_k[0, c].T).astype(bf)      # [128, 3072]
        pv = _pack_ktiles(np.ascontiguousarray(past_v[0, c])).astype(bf)
        wo_c = _pack_ktiles(
            np.ascontiguousarray(Wo[:, qs].T)).astype(bf)          # [128, 4*4096]
        in_maps.append({
            "hst": hst, "wqt": wq_c, "wkvt": wkv_c, "pastkt": pkt,
            "pastv": pv, "maskt": mask_t, "cosq": cosq_t, "sinq": sinq_t,
            "cosk": cosk_t, "sink": sink_t, "prot": prot_t, "sel2": sel2_np,
            "wot": wo_c,
        })

    res = run_bass_kernel_spmd(nc, in_maps, list(range(NCORES)))
    LAST_RESULTS = res
    out = np.zeros((Q, HID), np.float32)
    for c in range(NCORES):
        out += np.asarray(res.results[c]["outp"], dtype=np.float32)
    return out.reshape(B, Q, HID)


# revision 4
# speedup vs baseline: 1.0334x; 1.0159x over previous
"""Llama GQA attention (B=1, Q=1024, PAST=3072, HID=4096, NH=32, NKV=8, HD=128)
tensor-parallel over heads across 8 NeuronCores.

Per core c: kv head c, query heads 4c..4c+3. Each core computes its partial
o_proj contribution [1024, 4096] in bf16; the host sums the 8 partials in f32.

v2 layout (vs v1): engineered to unload the DVE (v1 bottleneck: 441us busy).
  - q/k proj W-stationary: out is [d, seq] (born transposed, no PE transposes).
    RoPE rotate-half via a PE permutation matmul; combine = 3 DVE TT ops.
  - v proj hs-stationary: out is [seq, d] directly in attn lhsT layout.
  - scores land in bf16 PSUM supertiles [128, 2048] (2 kv tiles x 1024 q);
    exp is ONE fused ACTIVATE per supertile reading PSUM directly. No mask
    add except on the diagonal tiles (DVE, in-place in PSUM).
  - causal skip: kv tiles 28..31 only computed against queries 512..1023
    (one extra narrow-quad supertile); kv 24..27 masked only vs q 0..511,
    same [128,2048] mask pattern serves both diagonals.
  - softmax denom: DVE-primary / GpSimd (every 4th kv tile) accumulation,
    ones-matmul partition reduce accumulated in PSUM, reciprocal via
    reciprocal_approx_fast, broadcast via gpsimd partition_broadcast.
  - o_proj: PSUM->SBUF copies split Scalar/Vector, bf16 output partials.
"""

import math
import numpy as np
import ml_dtypes

import bass_rust
import concourse.bass as bass
import concourse.mybir as mybir
import concourse.tile as tile
from concourse.vector_clock import ScopedClock
from concourse.bass_utils import run_bass_kernel_spmd

# ---------------------------------------------------------------------------
# Workaround: walrus in this image rejects >1 sem wait on CTRL-class
# instructions (Drain/NoOp). TileContext's tail drain waits on every touched
# logical processor. Split the waits across preceding sync-engine nops.
MAX_WAITS = 1


def _split_waits(nc, inst):
    si = inst.ins.sync_info
    if si is None:
        return
    waits = list(si.on_wait)
    if len(waits) <= MAX_WAITS:
        return
    inst.ins.sync_info = bass_rust.SyncInfo(
        on_wait=waits[:MAX_WAITS], on_update=list(si.on_update)
    )
    rest = waits[MAX_WAITS:]
    while rest:
        extra = nc.sync.nop(nofuse=True)
        extra.ins.sync_info = bass_rust.SyncInfo(on_wait=rest[:MAX_WAITS], on_update=[])
        rest = rest[MAX_WAITS:]


def _drain_and_barrier_split(self, tick_clock, wait_clock):
    nc = self.nc
    carrier = nc.sync.nop(nofuse=True)
    wait_clock.add_sem_waits(carrier.ins, ScopedClock({None: tick_clock.global_clock}))
    _split_waits(nc, carrier)
    nc.sync.drain()
    nc.all_engine_barrier()
    popped = nc._tile_sem_poison_stack.pop()
    assert popped is self._sem_poison
    nc.clear_and_free_semaphores(list(self.sems.allocated().values()))
    nc.all_engine_barrier()


tile.TileContext._drain_and_barrier = _drain_and_barrier_split
# ---------------------------------------------------------------------------

# ---------------------------------------------------------------------------
# General wait-cap legalization: this walrus rejects instructions carrying
# more than a couple of sem waits. Post-process the BIR JSON: hoist overflow
# waits onto engine-matched NoOps inserted immediately before the offender
# (same engine queue -> same ordering semantics).
import json as _json

_CTRL_OPS = {"NoOp", "Drain", "EventSemaphore"}
_CAP_CTRL = 1
_CAP_OTHER = 1
_orig_to_json_bytes = bass.Bass.to_json_bytes


def _legalized_to_json_bytes(self, *a, **k):
    raw = _orig_to_json_bytes(self, *a, **k)
    m = _json.loads(raw)
    ctr = [0]
    changed = False
    for fn in m.get("functions", []):
        for blk in fn.get("blocks", []):
            insts = blk.get("instructions", [])
            out = []
            for ins in insts:
                si = ins.get("sync_info")
                if si:
                    waits = si.get("on_wait") or []
                    cap = _CAP_CTRL if ins.get("opcode") in _CTRL_OPS else _CAP_OTHER
                    if len(waits) > cap:
                        changed = True
                        rest = waits[:-cap]
                        si["on_wait"] = waits[-cap:]
                        while rest:
                            ctr[0] += 1
                            out.append({
                                "debug": ins.get("debug", 0),
                                "engine": ins["engine"],
                                "ins": [], "outs": [],
                                "name": f"{ins['name']}_lw{ctr[0]}",
                                "opcode": "NoOp",
                                "sync_info": {"on_wait": rest[:_CAP_CTRL],
                                              "on_update": []},
                            })
                            rest = rest[_CAP_CTRL:]
                out.append(ins)
            blk["instructions"] = out
    if not changed:
        return raw
    return _json.dumps(m).encode()


bass.Bass.to_json_bytes = _legalized_to_json_bytes
# ---------------------------------------------------------------------------


B, Q, PAST, HID = 1, 1024, 3072, 4096
NH, NKV, HD = 32, 8, 128
KV = PAST + Q           # 4096
NCORES = 8
HPC = NH // NCORES      # 4 query heads per core
ROPE_THETA = 10000.0
EXP_SHIFT = -20.0       # constant softmax shift (cancels exactly per row)

F32 = mybir.dt.float32
BF16 = mybir.dt.bfloat16

N_KT = KV // 128        # 32 kv tiles
N_HK = HID // 128       # 32 hid k-tiles
GRP = 512               # query group width (stage 1)
N_G = Q // GRP          # 2 groups
N_PV = PAST // 128      # 24 past-v tiles
N_SUP = 30              # stage-2 supertiles/head: 28 full-q kv tiles + 2 narrow pairs

LAST_RESULTS = None     # test harness reads exec_time_ns from here


def _build_program():
    nc = bass.Bass()
    hst = nc.declare_dram_parameter("hst", [128, N_HK, Q], BF16, isOutput=False)
    wqt = nc.declare_dram_parameter("wqt", [128, N_HK, HPC * 128], BF16, isOutput=False)
    wkvt = nc.declare_dram_parameter("wkvt", [128, N_HK, 256], BF16, isOutput=False)
    pastkt = nc.declare_dram_parameter("pastkt", [128, PAST], BF16, isOutput=False)
    pastv = nc.declare_dram_parameter("pastv", [128, PAST], BF16, isOutput=False)
    maskt = nc.declare_dram_parameter("maskt", [128, 2048], BF16, isOutput=False)
    # rope tables in [d, seq] layout; q tables pre-scaled by 1/sqrt(HD)
    cosq = nc.declare_dram_parameter("cosq", [128, Q], BF16, isOutput=False)
    sinq = nc.declare_dram_parameter("sinq", [128, Q], BF16, isOutput=False)
    cosk = nc.declare_dram_parameter("cosk", [128, Q], BF16, isOutput=False)
    sink = nc.declare_dram_parameter("sink", [128, Q], BF16, isOutput=False)
    prot = nc.declare_dram_parameter("prot", [128, 128], BF16, isOutput=False)
    sel2 = nc.declare_dram_parameter("sel2", [2, 256], F32, isOutput=False)
    wot = nc.declare_dram_parameter("wot", [128, HPC * HID], BF16, isOutput=False)
    outp = nc.declare_dram_parameter("outp", [Q, HID], BF16, isOutput=True)

    with tile.TileContext(nc) as tc:
        with (
            tc.tile_pool(name="const", bufs=1) as cpool,
            tc.tile_pool(name="kvres", bufs=1) as kvpool,
            tc.tile_pool(name="qt", bufs=1) as qtpool,
            tc.tile_pool(name="attn", bufs=1) as apool,
        ):
            # ones2a/b: lhsT for denominator partition-reduce; row-select into
            # a shared [2, 512] PSUM bank (row 0 = q-half A, row 1 = q-half B)
            ones2a = cpool.tile([128, 2], BF16)
            nc.vector.memset(ones2a[:], 0.0)
            nc.vector.memset(ones2a[:, 0:1], 1.0)
            ones2b = cpool.tile([128, 2], BF16)
            nc.vector.memset(ones2b[:], 0.0)
            nc.vector.memset(ones2b[:, 1:2], 1.0)
            # sel_a/b: lhsT selecting row 0/1 of rc [2, 512] and broadcasting
            # it across all 128 output partitions (DMA'd: partition-sliced
            # memset is rejected by the BIR verifier)
            sel2_sb = cpool.tile([2, 256], F32)
            sel_a = sel2_sb[:, 0:128]
            sel_b = sel2_sb[:, 128:256]
            shift_sb = cpool.tile([128, 1], F32)
            nc.vector.memset(shift_sb[:], EXP_SHIFT)
            prot_sb = cpool.tile([128, 128], BF16)
            mask_sb = cpool.tile([128, 2048], BF16)

            # K_T [128 d, KV] bf16; V packed [128 kv-sub, N_KT*128 d]
            kt_sb = kvpool.tile([128, KV], BF16)
            v_sb = kvpool.tile([128, N_KT * 128], BF16)

            # qT per head [128 d, Q] bf16; attnT per head [128 d, Q] bf16
            qt_sb = [qtpool.tile([128, Q], BF16, tag=f"qt{h}", name=f"qt{h}") for h in range(HPC)]
            at_sb = [apool.tile([128, Q], BF16, tag=f"at{h}", name=f"at{h}") for h in range(HPC)]

            # ---------------- stage 1: QKV projection + RoPE ----------------
            with (
                tc.tile_pool(name="hsw", bufs=1) as hspool,
                tc.tile_pool(name="rope", bufs=2) as rpool,
                tc.tile_pool(name="qkps", bufs=1, space="PSUM") as qkps,
                tc.tile_pool(name="vps", bufs=2, space="PSUM") as vps,
                tc.tile_pool(name="rotps", bufs=1, space="PSUM") as rotps,
            ):
                hs_sb = hspool.tile([128, N_HK, Q], BF16)
                wq_sb = hspool.tile([128, N_HK, HPC * 128], BF16)
                wkv_sb = hspool.tile([128, N_HK, 256], BF16)
                cosq_sb = hspool.tile([128, Q], BF16)
                sinq_sb = hspool.tile([128, Q], BF16)
                cosk_sb = hspool.tile([128, Q], BF16)
                sink_sb = hspool.tile([128, Q], BF16)
                # stage-1-critical loads first, chunked for early start
                for i in range(8):
                    s, e = i * (N_HK // 8), (i + 1) * (N_HK // 8)
                    nc.sync.dma_start(hs_sb[:, s:e, :], hst[:, s:e, :])
                    nc.sync.dma_start(wq_sb[:, s:e, :], wqt[:, s:e, :])
                    if i < 4:
                        s2, e2 = i * (N_HK // 4), (i + 1) * (N_HK // 4)
                        nc.sync.dma_start(wkv_sb[:, s2:e2, :], wkvt[:, s2:e2, :])
                nc.sync.dma_start(cosq_sb[:], cosq[:])
                nc.sync.dma_start(sinq_sb[:], sinq[:])
                nc.sync.dma_start(cosk_sb[:], cosk[:])
                nc.sync.dma_start(sink_sb[:], sink[:])
                nc.sync.dma_start(prot_sb[:], prot[:])
                nc.sync.dma_start(mask_sb[:], maskt[:])
                nc.sync.dma_start(sel2_sb[:], sel2[:])
                nc.sync.dma_start(kt_sb[:, :PAST], pastkt[:])
                nc.sync.dma_start(v_sb[:, : N_PV * 128], pastv[:])

                def rope(dst_bf, src_ps, cos_t, sin_t, g):
                    """dst_bf [128 d, 512 s] <- RoPE applied in [d, s] layout.

                    rot = P_rot.T @ src (PE permutation matmul, sign folded
                    into P_rot); dst = src*cos + rot*sin.
                    """
                    c = cos_t[:, g * GRP:(g + 1) * GRP]
                    s = sin_t[:, g * GRP:(g + 1) * GRP]
                    q_f = rpool.tile([128, GRP], BF16, tag="qf", name="q_f")
                    nc.scalar.copy(q_f[:], src_ps[:])
                    rot_ps = rotps.tile([128, GRP], F32, tag="rot", name="rot_ps")
                    nc.tensor.matmul(rot_ps[:], prot_sb[:], q_f[:],
                                     start=True, stop=True)
                    t1 = rpool.tile([128, GRP], F32, tag="t1", name="t1")
                    nc.vector.tensor_mul(t1[:], src_ps[:], c)
                    t2 = rpool.tile([128, GRP], F32, tag="t2", name="t2")
                    nc.vector.tensor_mul(t2[:], rot_ps[:], s)
                    nc.vector.tensor_add(dst_bf, t1[:], t2[:])

                for g in range(N_G):
                    gsl = slice(g * GRP, (g + 1) * GRP)
                    q_ps = [qkps.tile([128, GRP], F32, tag=f"qps{h}", name=f"qps{h}")
                            for h in range(HPC)]
                    k_ps = qkps.tile([128, GRP], F32, tag="kps", name="k_ps")
                    for k in range(N_HK):
                        rhs = hs_sb[:, k:k + 1, g * GRP:(g + 1) * GRP]
                        for h in range(HPC):
                            nc.tensor.matmul(
                                q_ps[h][:],
                                wq_sb[:, k:k + 1, h * 128:(h + 1) * 128],
                                rhs, start=(k == 0), stop=(k == N_HK - 1),
                            )
                        nc.tensor.matmul(
                            k_ps[:], wkv_sb[:, k:k + 1, 0:128], rhs,
                            start=(k == 0), stop=(k == N_HK - 1),
                        )
                    # v proj: hs-stationary, out [seq, d] per 128-seq tile
                    for st in range(4):
                        gst = g * 4 + st
                        v_ps = vps.tile([128, 128], F32, tag="vp", name="v_ps")
                        for k in range(N_HK):
                            nc.tensor.matmul(
                                v_ps[:],
                                hs_sb[:, k:k + 1, gst * 128:(gst + 1) * 128],
                                wkv_sb[:, k:k + 1, 128:256],
                                start=(k == 0), stop=(k == N_HK - 1),
                            )
                        nc.scalar.copy(
                            v_sb[:, (N_PV + gst) * 128:(N_PV + gst + 1) * 128],
                            v_ps[:],
                        )
                    for h in range(HPC):
                        rope(qt_sb[h][:, gsl], q_ps[h], cosq_sb, sinq_sb, g)
                    rope(kt_sb[:, PAST + g * GRP: PAST + (g + 1) * GRP],
                         k_ps, cosk_sb, sink_sb, g)

            # ------------- stage 2 + 3 (wo loads during stage 2) -------------
            with (
                tc.tile_pool(name="wo", bufs=1) as wopool,
                tc.tile_pool(name="ostage", bufs=2) as ostpool,
            ):
                wo_sb = wopool.tile([128, HPC * HID], BF16)
                for h in range(HPC):
                    nc.sync.dma_start(
                        wo_sb[:, h * HID:(h + 1) * HID],
                        wot[:, h * HID:(h + 1) * HID],
                    )

                # ---------------- stage 2: attention ----------------
                with (
                    tc.tile_pool(name="pt", bufs=6) as ptpool,
                    tc.tile_pool(name="softm", bufs=2) as smpool,
                    tc.tile_pool(name="scps", bufs=2, space="PSUM") as scps,
                    tc.tile_pool(name="aps", bufs=1, space="PSUM") as aps,
                    tc.tile_pool(name="dps", bufs=1, space="PSUM") as dps,
                ):
                    deferred_tail = [[]]

                    for h in range(HPC):
                        a_ps = aps.tile([128, Q], F32, tag="aacc", name="a_ps")
                        # denominator partials on DVE in bf16 (2 accumulators
                        # -> rounding error ~sqrt(15)*2^-9, well within budget)
                        dn0 = smpool.tile([128, Q], BF16, tag="dn0", name="dn0")
                        dn1 = smpool.tile([128, Q], BF16, tag="dn1", name="dn1")

                        def emit_attn(prev):
                            """Attn accumulation for the previous supertile
                            (lagged so the PE never waits on this supertile's
                            exp)."""
                            _, pt, plan = prev
                            for (kt, col, qoff) in plan:
                                nc.tensor.matmul(
                                    a_ps[:, qoff:qoff + GRP],
                                    v_sb[:, kt * 128:(kt + 1) * 128],
                                    pt[:, col:col + GRP],
                                    start=(kt == 0),
                                    stop=(kt == 27 if qoff == 0 else kt == 31),
                                )

                        def dn_accum(jj, pt, plan):
                            if jj < 28:
                                dn = dn0 if jj % 2 == 0 else dn1
                                if jj < 2:
                                    nc.vector.tensor_copy(dn[:], pt[:])
                                else:
                                    nc.vector.tensor_add(dn[:], dn[:], pt[:])
                            else:
                                for (kt, col, qoff) in plan:
                                    dn = dn0 if kt % 2 == 0 else dn1
                                    nc.vector.tensor_add(
                                        dn[:, qoff:qoff + GRP],
                                        dn[:, qoff:qoff + GRP],
                                        pt[:, col:col + GRP],
                                    )

                        pend = []
                        for jj in range(N_SUP):
                            s_sup = scps.tile([128, 1024], F32, tag="ss", name="s_sup")
                            pt = ptpool.tile([128, 1024], BF16, tag="pt", name="pt")
                            plan = []
                            if jj < 28:
                                # kv tile jj x full q
                                kt = jj
                                for ii in range(2):
                                    nc.tensor.matmul(
                                        s_sup[:, ii * GRP:(ii + 1) * GRP],
                                        kt_sb[:, kt * 128:(kt + 1) * 128],
                                        qt_sb[h][:, ii * GRP:(ii + 1) * GRP],
                                        start=True, stop=True,
                                    )
                                    plan.append((kt, ii * GRP, ii * GRP))
                            else:
                                # narrow pair: kv (28,29) or (30,31) x q-half B
                                for c in range(2):
                                    kt = 28 + 2 * (jj - 28) + c
                                    nc.tensor.matmul(
                                        s_sup[:, c * GRP:(c + 1) * GRP],
                                        kt_sb[:, kt * 128:(kt + 1) * 128],
                                        qt_sb[h][:, GRP:Q],
                                        start=True, stop=True,
                                    )
                                    plan.append((kt, c * GRP, GRP))
                            nc.scalar.activation(
                                pt[:], s_sup[:],
                                mybir.ActivationFunctionType.Exp,
                                bias=shift_sb[:], scale=1.0,
                            )
                            if jj >= 24:
                                if jj < 28:
                                    nc.gpsimd.tensor_mul(
                                        pt[:, 0:GRP], pt[:, 0:GRP],
                                        mask_sb[:, (jj - 24) * GRP:(jj - 23) * GRP],
                                    )
                                else:
                                    for (kt, col, qoff) in plan:
                                        nc.gpsimd.tensor_mul(
                                            pt[:, col:col + GRP],
                                            pt[:, col:col + GRP],
                                            mask_sb[:, (kt - 28) * GRP:(kt - 27) * GRP],
                                        )
                            dn_accum(jj, pt, plan)
                            pend.append((jj, pt, plan))
                            if len(pend) > 2:
                                emit_attn(pend.pop(0))
                            if jj in (2, 8) and deferred_tail[0]:
                                deferred_tail[0].pop(0)()
                        for ent in pend:
                            emit_attn(ent)
                        # copy a_ps out unnormalized right away (frees a_ps
                        # for the next head); the whole denominator/normalize
                        # tail is deferred into the next head's loop so its
                        # PE ops never block the next head's scores while
                        # waiting on the DVE dn-drain/reciprocal
                        au_sb = smpool.tile([128, Q], BF16, tag="atu", name="au_sb")
                        nc.vector.tensor_copy(au_sb[:], a_ps[:])

                        def make_tail(h, au_sb, dn0, dn1):
                            rc_sb = smpool.tile([2, GRP], F32, tag="recip",
                                                name="rc_sb")

                            def tail_ds():
                                ds_ps = dps.tile([2, GRP], F32, tag="dsum",
                                                 name="ds_ps")
                                for idx, (sel, dn, hoff) in enumerate(
                                    [(ones2a, dn0, 0), (ones2a, dn1, 0),
                                     (ones2b, dn0, GRP), (ones2b, dn1, GRP)]
                                ):
                                    nc.tensor.matmul(
                                        ds_ps[:], sel[:], dn[:, hoff:hoff + GRP],
                                        start=(idx == 0), stop=(idx == 3),
                                    )
                                nc.vector.reciprocal(rc_sb[:], ds_ps[:])

                            def tail_bc():
                                bc_sb = smpool.tile([128, Q], F32, tag="bcast",
                                                    name="bc_sb")
                                for half in range(2):
                                    hsl = slice(half * GRP, (half + 1) * GRP)
                                    bc_ps = dps.tile([128, GRP], F32, tag="bcps",
                                                     name="bc_ps")
                                    nc.tensor.matmul(
                                        bc_ps[:], sel_a if half == 0 else sel_b,
                                        rc_sb[:], start=True, stop=True)
                                    nc.vector.tensor_copy(bc_sb[:, hsl], bc_ps[:])
                                nc.vector.tensor_mul(at_sb[h][:], au_sb[:],
                                                     bc_sb[:])
                            return [tail_ds, tail_bc]

                        deferred_tail[0] = make_tail(h, au_sb, dn0, dn1)
                    for fn in deferred_tail[0]:
                        fn()

                # ---------------- stage 3: o_proj partial ----------------
                with tc.tile_pool(name="ops", bufs=2, space="PSUM") as opps:
                    for st in range(8):
                        for half in range(2):
                            o_sb = ostpool.tile([128, 2048], BF16, tag="osb",
                                                name="o_sb")
                            o_ps = opps.tile([128, 2048], F32, tag="ops",
                                             name="o_ps")
                            for h in range(HPC):
                                for nn in range(4):
                                    n = half * 4 + nn
                                    nc.tensor.matmul(
                                        o_ps[:, nn * 512:(nn + 1) * 512],
                                        at_sb[h][:, st * 128:(st + 1) * 128],
                                        wo_sb[:, h * HID + n * 512:
                                              h * HID + (n + 1) * 512],
                                        start=(h == 0), stop=(h == HPC - 1),
                                    )
                            if (st + half) % 2 == 0:
                                nc.scalar.copy(o_sb[:], o_ps[:])
                            else:
                                nc.vector.tensor_copy(o_sb[:], o_ps[:])
                            nc.sync.dma_start(
                                outp[st * 128:(st + 1) * 128,
                                     half * 2048:(half + 1) * 2048],
                                o_sb[:],
                            )
    return nc


def _pack_ktiles(a, tile_rows=128):
    """[R, C] -> [128, (R//128)*C] with k-tile kt at cols [kt*C:(kt+1)*C]."""
    r, c = a.shape
    n = r // tile_rows
    return np.ascontiguousarray(
        a.reshape(n, tile_rows, c).transpose(1, 0, 2).reshape(tile_rows, n * c)
    )


def _rope_tables_ds(position_ids):
    """cos/sin in [d, s] layout: [128, Q] f64."""
    pos = np.asarray(position_ids).reshape(-1).astype(np.float64)      # [Q]
    inv_freq = 1.0 / (ROPE_THETA ** (np.arange(0, HD, 2, dtype=np.float64) / HD))
    ang_half = np.outer(inv_freq, pos)                                 # [64, Q]
    ang = np.concatenate([ang_half, ang_half], axis=0)                 # [128, Q]
    return np.cos(ang), np.sin(ang)


def kernel(hidden_states, attention_mask, position_ids, past_k, past_v,
           Wq, Wk, Wv, Wo):
    global LAST_RESULTS
    bf = ml_dtypes.bfloat16

    hs = np.asarray(hidden_states, np.float32).reshape(Q, HID)
    mask = np.asarray(attention_mask, np.float32).reshape(Q, KV)
    cos_d, sin_d = _rope_tables_ds(position_ids)

    scale = 1.0 / math.sqrt(HD)
    cosq_t = (cos_d * scale).astype(bf)
    sinq_t = (sin_d * scale).astype(bf)
    cosk_t = cos_d.astype(bf)
    sink_t = sin_d.astype(bf)

    # rotate-half permutation with sign: rot[d] = -x[d+64] (d<64); x[d-64]
    prot_np = np.zeros((128, 128), np.float32)
    for dd in range(64):
        prot_np[dd + 64, dd] = -1.0     # lhsT[d', d]: rot[d] += P[d', d] * x[d']
        prot_np[dd, dd + 64] = 1.0
    prot_t = prot_np.astype(bf)

    # diagonal masks: [128 kv, 4 tiles * 512 q]: kv tile 24+m vs queries
    # 0..511 (identical pattern to kv tile 28+m vs queries 512..1023)
    mask_t = np.empty((128, 2048), np.float32)
    for m in range(4):
        kt = 24 + m
        blk = mask[0:512, kt * 128:(kt + 1) * 128].T
        mask_t[:, m * 512:(m + 1) * 512] = (blk == 0.0).astype(np.float32)
    mask_t = mask_t.astype(bf)

    sel2_np = np.zeros((2, 256), np.float32)
    sel2_np[0, 0:128] = 1.0      # sel_a: broadcast rc row 0
    sel2_np[1, 128:256] = 1.0    # sel_b: broadcast rc row 1

    hst = _pack_ktiles(np.ascontiguousarray(hs.T)).astype(bf)      # [128, 32, 1024]
    hst = hst.reshape(128, N_HK, Q)

    nc = _build_program()
    in_maps = []
    for c in range(NCORES):
        qs = slice(c * HPC * HD, (c + 1) * HPC * HD)
        ks = slice(c * HD, (c + 1) * HD)
        wq_c = _pack_ktiles(
            np.ascontiguousarray(Wq[qs, :].T)
        ).astype(bf).reshape(128, N_HK, HPC * 128)
        wk_c = np.ascontiguousarray(Wk[ks, :].T)                   # [4096, 128]
        wv_c = np.ascontiguousarray(Wv[ks, :].T)
        wkv_c = _pack_ktiles(
            np.concatenate([wk_c, wv_c], axis=1)
        ).astype(bf).reshape(128, N_HK, 256)
        pkt = np.ascontiguousarray(past_k[0, c].T).astype(bf)      # [128, 3072]
        pv = _pack_ktiles(np.ascontiguousarray(past_v[0, c])).astype(bf)
        wo_c = _pack_ktiles(
            np.ascontiguousarray(Wo[:, qs].T)).astype(bf)          # [128, 4*4096]
        in_maps.append({
            "hst": hst, "wqt": wq_c, "wkvt": wkv_c, "pastkt": pkt,
            "pastv": pv, "maskt": mask_t, "cosq": cosq_t, "sinq": sinq_t,
            "cosk": cosk_t, "sink": sink_t, "prot": prot_t, "sel2": sel2_np,
            "wot": wo_c,
        })

    res = run_bass_kernel_spmd(nc, in_maps, list(range(NCORES)))
    LAST_RESULTS = res
    out = np.zeros((Q, HID), np.float32)
    for c in range(NCORES):
        out += np.asarray(res.results[c]["outp"], dtype=np.float32)
    return out.reshape(B, Q, HID)


# revision 5
# speedup vs baseline: 1.0659x; 1.0314x over previous
"""Llama GQA attention (B=1, Q=1024, PAST=3072, HID=4096, NH=32, NKV=8, HD=128)
tensor-parallel over heads across 8 NeuronCores.

Per core c: kv head c, query heads 4c..4c+3. Each core computes its partial
o_proj contribution [1024, 4096] in bf16; the host sums the 8 partials in f32.

v2 layout (vs v1): engineered to unload the DVE (v1 bottleneck: 441us busy).
  - q/k proj W-stationary: out is [d, seq] (born transposed, no PE transposes).
    RoPE rotate-half via a PE permutation matmul; combine = 3 DVE TT ops.
  - v proj hs-stationary: out is [seq, d] directly in attn lhsT layout.
  - scores land in bf16 PSUM supertiles [128, 2048] (2 kv tiles x 1024 q);
    exp is ONE fused ACTIVATE per supertile reading PSUM directly. No mask
    add except on the diagonal tiles (DVE, in-place in PSUM).
  - causal skip: kv tiles 28..31 only computed against queries 512..1023
    (one extra narrow-quad supertile); kv 24..27 masked only vs q 0..511,
    same [128,2048] mask pattern serves both diagonals.
  - softmax denom: DVE-primary / GpSimd (every 4th kv tile) accumulation,
    ones-matmul partition reduce accumulated in PSUM, reciprocal via
    reciprocal_approx_fast, broadcast via gpsimd partition_broadcast.
  - o_proj: PSUM->SBUF copies split Scalar/Vector, bf16 output partials.
"""

import math
import numpy as np
import ml_dtypes

import bass_rust
import concourse.bass as bass
import concourse.mybir as mybir
import concourse.tile as tile
from concourse.vector_clock import ScopedClock
from concourse.bass_utils import run_bass_kernel_spmd

# ---------------------------------------------------------------------------
# Workaround: walrus in this image rejects >1 sem wait on CTRL-class
# instructions (Drain/NoOp). TileContext's tail drain waits on every touched
# logical processor. Split the waits across preceding sync-engine nops.
MAX_WAITS = 1


def _split_waits(nc, inst):
    si = inst.ins.sync_info
    if si is None:
        return
    waits = list(si.on_wait)
    if len(waits) <= MAX_WAITS:
        return
    inst.ins.sync_info = bass_rust.SyncInfo(
        on_wait=waits[:MAX_WAITS], on_update=list(si.on_update)
    )
    rest = waits[MAX_WAITS:]
    while rest:
        extra = nc.sync.nop(nofuse=True)
        extra.ins.sync_info = bass_rust.SyncInfo(on_wait=rest[:MAX_WAITS], on_update=[])
        rest = rest[MAX_WAITS:]


def _drain_and_barrier_split(self, tick_clock, wait_clock):
    nc = self.nc
    carrier = nc.sync.nop(nofuse=True)
    wait_clock.add_sem_waits(carrier.ins, ScopedClock({None: tick_clock.global_clock}))
    _split_waits(nc, carrier)
    nc.sync.drain()
    nc.all_engine_barrier()
    popped = nc._tile_sem_poison_stack.pop()
    assert popped is self._sem_poison
    nc.clear_and_free_semaphores(list(self.sems.allocated().values()))
    nc.all_engine_barrier()


tile.TileContext._drain_and_barrier = _drain_and_barrier_split
# ---------------------------------------------------------------------------

# ---------------------------------------------------------------------------
# General wait-cap legalization: this walrus rejects instructions carrying
# more than a couple of sem waits. Post-process the BIR JSON: hoist overflow
# waits onto engine-matched NoOps inserted immediately before the offender
# (same engine queue -> same ordering semantics).
import json as _json

_CTRL_OPS = {"NoOp", "Drain", "EventSemaphore"}
_CAP_CTRL = 1
_CAP_OTHER = 1
_orig_to_json_bytes = bass.Bass.to_json_bytes


def _legalized_to_json_bytes(self, *a, **k):
    raw = _orig_to_json_bytes(self, *a, **k)
    m = _json.loads(raw)
    ctr = [0]
    changed = False
    for fn in m.get("functions", []):
        for blk in fn.get("blocks", []):
            insts = blk.get("instructions", [])
            out = []
            for ins in insts:
                si = ins.get("sync_info")
                if si:
                    waits = si.get("on_wait") or []
                    cap = _CAP_CTRL if ins.get("opcode") in _CTRL_OPS else _CAP_OTHER
                    if len(waits) > cap:
                        changed = True
                        rest = waits[:-cap]
                        si["on_wait"] = waits[-cap:]
                        while rest:
                            ctr[0] += 1
                            out.append({
                                "debug": ins.get("debug", 0),
                                "engine": ins["engine"],
                                "ins": [], "outs": [],
                                "name": f"{ins['name']}_lw{ctr[0]}",
                                "opcode": "NoOp",
                                "sync_info": {"on_wait": rest[:_CAP_CTRL],
                                              "on_update": []},
                            })
                            rest = rest[_CAP_CTRL:]
                out.append(ins)
            blk["instructions"] = out
    if not changed:
        return raw
    return _json.dumps(m).encode()


bass.Bass.to_json_bytes = _legalized_to_json_bytes
# ---------------------------------------------------------------------------


B, Q, PAST, HID = 1, 1024, 3072, 4096
NH, NKV, HD = 32, 8, 128
KV = PAST + Q           # 4096
NCORES = 8
HPC = NH // NCORES      # 4 query heads per core
ROPE_THETA = 10000.0
EXP_SHIFT = -20.0       # constant softmax shift (cancels exactly per row)

F32 = mybir.dt.float32
BF16 = mybir.dt.bfloat16

N_KT = KV // 128        # 32 kv tiles
N_HK = HID // 128       # 32 hid k-tiles
GRP = 512               # query group width (stage 1)
N_G = Q // GRP          # 2 groups
N_PV = PAST // 128      # 24 past-v tiles
N_SUP = 30              # stage-2 supertiles/head: 28 full-q kv tiles + 2 narrow pairs

LAST_RESULTS = None     # test harness reads exec_time_ns from here


def _build_program():
    nc = bass.Bass()
    hst = nc.declare_dram_parameter("hst", [128, N_HK, Q], BF16, isOutput=False)
    wqt = nc.declare_dram_parameter("wqt", [128, N_HK, HPC * 128], BF16, isOutput=False)
    wkvt = nc.declare_dram_parameter("wkvt", [128, N_HK, 256], BF16, isOutput=False)
    pastkt = nc.declare_dram_parameter("pastkt", [128, PAST], BF16, isOutput=False)
    pastv = nc.declare_dram_parameter("pastv", [128, PAST], BF16, isOutput=False)
    maskt = nc.declare_dram_parameter("maskt", [128, 2048], BF16, isOutput=False)
    # rope tables in [d, seq] layout; q tables pre-scaled by 1/sqrt(HD)
    cosq = nc.declare_dram_parameter("cosq", [128, Q], BF16, isOutput=False)
    sinq = nc.declare_dram_parameter("sinq", [128, Q], BF16, isOutput=False)
    cosk = nc.declare_dram_parameter("cosk", [128, Q], BF16, isOutput=False)
    sink = nc.declare_dram_parameter("sink", [128, Q], BF16, isOutput=False)
    prot = nc.declare_dram_parameter("prot", [128, 128], BF16, isOutput=False)
    sel2 = nc.declare_dram_parameter("sel2", [2, 256], F32, isOutput=False)
    wot = nc.declare_dram_parameter("wot", [128, HPC * HID], BF16, isOutput=False)
    outp = nc.declare_dram_parameter("outp", [Q, HID], BF16, isOutput=True)

    with tile.TileContext(nc) as tc:
        with (
            tc.tile_pool(name="const", bufs=1) as cpool,
            tc.tile_pool(name="kvres", bufs=1) as kvpool,
            tc.tile_pool(name="qt", bufs=1) as qtpool,
            tc.tile_pool(name="attn", bufs=1) as apool,
        ):
            # ones2a/b: lhsT for denominator partition-reduce; row-select into
            # a shared [2, 512] PSUM bank (row 0 = q-half A, row 1 = q-half B)
            ones2a = cpool.tile([128, 2], BF16)
            nc.vector.memset(ones2a[:], 0.0)
            nc.vector.memset(ones2a[:, 0:1], 1.0)
            ones2b = cpool.tile([128, 2], BF16)
            nc.vector.memset(ones2b[:], 0.0)
            nc.vector.memset(ones2b[:, 1:2], 1.0)
            # sel_a/b: lhsT selecting row 0/1 of rc [2, 512] and broadcasting
            # it across all 128 output partitions (DMA'd: partition-sliced
            # memset is rejected by the BIR verifier)
            sel2_sb = cpool.tile([2, 256], F32)
            sel_a = sel2_sb[:, 0:128]
            sel_b = sel2_sb[:, 128:256]
            shift_sb = cpool.tile([128, 1], F32)
            nc.vector.memset(shift_sb[:], EXP_SHIFT)
            prot_sb = cpool.tile([128, 128], BF16)
            mask_sb = cpool.tile([128, 2048], BF16)

            # K_T [128 d, KV] bf16; V packed [128 kv-sub, N_KT*128 d]
            kt_sb = kvpool.tile([128, KV], BF16)
            v_sb = kvpool.tile([128, N_KT * 128], BF16)

            # qT per head [128 d, Q] bf16; attnT per head [128 d, Q] bf16
            qt_sb = [qtpool.tile([128, Q], BF16, tag=f"qt{h}", name=f"qt{h}") for h in range(HPC)]
            at_sb = [apool.tile([128, Q], BF16, tag=f"at{h}", name=f"at{h}") for h in range(HPC)]

            # ---------------- stage 1: QKV projection + RoPE ----------------
            with (
                tc.tile_pool(name="hsw", bufs=1) as hspool,
                tc.tile_pool(name="rope", bufs=2) as rpool,
                tc.tile_pool(name="qkps", bufs=1, space="PSUM") as qkps,
                tc.tile_pool(name="vps", bufs=2, space="PSUM") as vps,
                tc.tile_pool(name="rotps", bufs=1, space="PSUM") as rotps,
            ):
                hs_sb = hspool.tile([128, N_HK, Q], BF16)
                wq_sb = hspool.tile([128, N_HK, HPC * 128], BF16)
                wkv_sb = hspool.tile([128, N_HK, 256], BF16)
                cosq_sb = hspool.tile([128, Q], BF16)
                sinq_sb = hspool.tile([128, Q], BF16)
                cosk_sb = hspool.tile([128, Q], BF16)
                sink_sb = hspool.tile([128, Q], BF16)
                # stage-1-critical loads first, chunked for early start
                for i in range(8):
                    s, e = i * (N_HK // 8), (i + 1) * (N_HK // 8)
                    nc.sync.dma_start(hs_sb[:, s:e, :], hst[:, s:e, :])
                    nc.sync.dma_start(wq_sb[:, s:e, :], wqt[:, s:e, :])
                    if i < 4:
                        s2, e2 = i * (N_HK // 4), (i + 1) * (N_HK // 4)
                        nc.sync.dma_start(wkv_sb[:, s2:e2, :], wkvt[:, s2:e2, :])
                nc.sync.dma_start(cosq_sb[:], cosq[:])
                nc.sync.dma_start(sinq_sb[:], sinq[:])
                nc.sync.dma_start(cosk_sb[:], cosk[:])
                nc.sync.dma_start(sink_sb[:], sink[:])
                nc.sync.dma_start(prot_sb[:], prot[:])
                nc.sync.dma_start(mask_sb[:], maskt[:])
                nc.sync.dma_start(sel2_sb[:], sel2[:])
                nc.sync.dma_start(kt_sb[:, :PAST], pastkt[:])
                nc.sync.dma_start(v_sb[:, : N_PV * 128], pastv[:])

                def rope(dst_bf, src_ps, cos_t, sin_t, g):
                    """dst_bf [128 d, 512 s] <- RoPE applied in [d, s] layout.

                    rot = P_rot.T @ src (PE permutation matmul, sign folded
                    into P_rot); dst = src*cos + rot*sin.
                    """
                    c = cos_t[:, g * GRP:(g + 1) * GRP]
                    s = sin_t[:, g * GRP:(g + 1) * GRP]
                    q_f = rpool.tile([128, GRP], BF16, tag="qf", name="q_f")
                    nc.scalar.copy(q_f[:], src_ps[:])
                    rot_ps = rotps.tile([128, GRP], F32, tag="rot", name="rot_ps")
                    nc.tensor.matmul(rot_ps[:], prot_sb[:], q_f[:],
                                     start=True, stop=True)
                    t1 = rpool.tile([128, GRP], F32, tag="t1", name="t1")
                    nc.vector.tensor_mul(t1[:], src_ps[:], c)
                    t2 = rpool.tile([128, GRP], F32, tag="t2", name="t2")
                    nc.vector.tensor_mul(t2[:], rot_ps[:], s)
                    nc.vector.tensor_add(dst_bf, t1[:], t2[:])

                for g in range(N_G):
                    gsl = slice(g * GRP, (g + 1) * GRP)
                    q_ps = [qkps.tile([128, GRP], F32, tag=f"qps{h}", name=f"qps{h}")
                            for h in range(HPC)]
                    k_ps = qkps.tile([128, GRP], F32, tag="kps", name="k_ps")
                    for k in range(N_HK):
                        rhs = hs_sb[:, k:k + 1, g * GRP:(g + 1) * GRP]
                        for h in range(HPC):
                            nc.tensor.matmul(
                                q_ps[h][:],
                                wq_sb[:, k:k + 1, h * 128:(h + 1) * 128],
                                rhs, start=(k == 0), stop=(k == N_HK - 1),
                            )
                        nc.tensor.matmul(
                            k_ps[:], wkv_sb[:, k:k + 1, 0:128], rhs,
                            start=(k == 0), stop=(k == N_HK - 1),
                        )
                    # v proj: hs-stationary, out [seq, d] per 128-seq tile
                    for st in range(4):
                        gst = g * 4 + st
                        v_ps = vps.tile([128, 128], F32, tag="vp", name="v_ps")
                        for k in range(N_HK):
                            nc.tensor.matmul(
                                v_ps[:],
                                hs_sb[:, k:k + 1, gst * 128:(gst + 1) * 128],
                                wkv_sb[:, k:k + 1, 128:256],
                                start=(k == 0), stop=(k == N_HK - 1),
                            )
                        nc.scalar.copy(
                            v_sb[:, (N_PV + gst) * 128:(N_PV + gst + 1) * 128],
                            v_ps[:],
                        )
                    for h in range(HPC):
                        rope(qt_sb[h][:, gsl], q_ps[h], cosq_sb, sinq_sb, g)
                    rope(kt_sb[:, PAST + g * GRP: PAST + (g + 1) * GRP],
                         k_ps, cosk_sb, sink_sb, g)

            # ------------- stage 2 + 3 (wo loads during stage 2) -------------
            with (
                tc.tile_pool(name="wo", bufs=1) as wopool,
                tc.tile_pool(name="ostage", bufs=2) as ostpool,
            ):
                wo_sb = wopool.tile([128, HPC * HID], BF16)
                for h in range(HPC):
                    nc.sync.dma_start(
                        wo_sb[:, h * HID:(h + 1) * HID],
                        wot[:, h * HID:(h + 1) * HID],
                    )

                # ---------------- stage 2: attention ----------------
                with (
                    tc.tile_pool(name="pt", bufs=6) as ptpool,
                    tc.tile_pool(name="softm", bufs=2) as smpool,
                    tc.tile_pool(name="scps", bufs=2, space="PSUM") as scps,
                    tc.tile_pool(name="aps", bufs=1, space="PSUM") as aps,
                    tc.tile_pool(name="dps", bufs=1, space="PSUM") as dps,
                ):
                    deferred_tail = [[]]

                    for h in range(HPC):
                        a_ps = aps.tile([128, Q], F32, tag="aacc", name="a_ps")
                        # denominator partials on DVE in bf16 (2 accumulators
                        # -> rounding error ~sqrt(15)*2^-9, well within budget)
                        dn0 = smpool.tile([128, Q], BF16, tag="dn0", name="dn0")
                        dn1 = smpool.tile([128, Q], BF16, tag="dn1", name="dn1")

                        def emit_attn(prev):
                            """Attn accumulation for the previous supertile
                            (lagged so the PE never waits on this supertile's
                            exp)."""
                            _, pt, plan = prev
                            for (kt, col, qoff) in plan:
                                nc.tensor.matmul(
                                    a_ps[:, qoff:qoff + GRP],
                                    v_sb[:, kt * 128:(kt + 1) * 128],
                                    pt[:, col:col + GRP],
                                    start=(kt == 24),
                                    stop=(kt == 23),
                                )

                        def dn_accum(jj, pt, plan):
                            if jj < 28:
                                key = jj % 2
                                dn = dn0 if key == 0 else dn1
                                if key not in touched:
                                    touched.add(key)
                                    nc.vector.tensor_copy(dn[:], pt[:])
                                else:
                                    nc.vector.tensor_add(dn[:], dn[:], pt[:])
                            else:
                                for (kt, col, qoff) in plan:
                                    dn = dn0 if kt % 2 == 0 else dn1
                                    nc.vector.tensor_add(
                                        dn[:, qoff:qoff + GRP],
                                        dn[:, qoff:qoff + GRP],
                                        pt[:, col:col + GRP],
                                    )

                        pend = []
                        touched = set()
                        # masked supertiles first, interleaved 1:1 with
                        # unmasked ones (gp mask-muls spread over ~13us);
                        # PSUM accumulation is order-independent. dn0/dn1
                        # first-touch stays on FULL supertiles (24, 25).
                        ORDER = [24, 0, 25, 1, 26, 2, 27, 3, 28, 4, 29, 5] + \
                            list(range(6, 24))
                        for pos, jj in enumerate(ORDER):
                            s_sup = scps.tile([128, 1024], F32, tag="ss", name="s_sup")
                            pt = ptpool.tile([128, 1024], BF16, tag="pt", name="pt")
                            plan = []
                            if jj < 28:
                                # kv tile jj x full q
                                kt = jj
                                for ii in range(2):
                                    nc.tensor.matmul(
                                        s_sup[:, ii * GRP:(ii + 1) * GRP],
                                        kt_sb[:, kt * 128:(kt + 1) * 128],
                                        qt_sb[h][:, ii * GRP:(ii + 1) * GRP],
                                        start=True, stop=True,
                                    )
                                    plan.append((kt, ii * GRP, ii * GRP))
                            else:
                                # narrow pair: kv (28,29) or (30,31) x q-half B
                                for c in range(2):
                                    kt = 28 + 2 * (jj - 28) + c
                                    nc.tensor.matmul(
                                        s_sup[:, c * GRP:(c + 1) * GRP],
                                        kt_sb[:, kt * 128:(kt + 1) * 128],
                                        qt_sb[h][:, GRP:Q],
                                        start=True, stop=True,
                                    )
                                    plan.append((kt, c * GRP, GRP))
                            nc.scalar.activation(
                                pt[:], s_sup[:],
                                mybir.ActivationFunctionType.Exp,
                                bias=shift_sb[:], scale=1.0,
                            )
                            if jj >= 24:
                                if jj < 28:
                                    nc.gpsimd.tensor_mul(
                                        pt[:, 0:GRP], pt[:, 0:GRP],
                                        mask_sb[:, (jj - 24) * GRP:(jj - 23) * GRP],
                                    )
                                else:
                                    for (kt, col, qoff) in plan:
                                        nc.gpsimd.tensor_mul(
                                            pt[:, col:col + GRP],
                                            pt[:, col:col + GRP],
                                            mask_sb[:, (kt - 28) * GRP:(kt - 27) * GRP],
                                        )
                            dn_accum(jj, pt, plan)
                            pend.append((jj, pt, plan))
                            if len(pend) > 2:
                                emit_attn(pend.pop(0))
                            if pos in (2, 8) and deferred_tail[0]:
                                deferred_tail[0].pop(0)()
                        for ent in pend:
                            emit_attn(ent)
                        # copy a_ps out unnormalized right away (frees a_ps
                        # for the next head); the whole denominator/normalize
                        # tail is deferred into the next head's loop so its
                        # PE ops never block the next head's scores while
                        # waiting on the DVE dn-drain/reciprocal
                        au_sb = smpool.tile([128, Q], BF16, tag="atu", name="au_sb")
                        nc.vector.tensor_copy(au_sb[:], a_ps[:])

                        def make_tail(h, au_sb, dn0, dn1):
                            rc_sb = smpool.tile([2, GRP], F32, tag="recip",
                                                name="rc_sb")

                            def tail_ds():
                                ds_ps = dps.tile([2, GRP], F32, tag="dsum",
                                                 name="ds_ps")
                                for idx, (sel, dn, hoff) in enumerate(
                                    [(ones2a, dn0, 0), (ones2a, dn1, 0),
                                     (ones2b, dn0, GRP), (ones2b, dn1, GRP)]
                                ):
                                    nc.tensor.matmul(
                                        ds_ps[:], sel[:], dn[:, hoff:hoff + GRP],
                                        start=(idx == 0), stop=(idx == 3),
                                    )
                                nc.vector.reciprocal(rc_sb[:], ds_ps[:])

                            def tail_bc():
                                bc_sb = smpool.tile([128, Q], F32, tag="bcast",
                                                    name="bc_sb")
                                for half in range(2):
                                    hsl = slice(half * GRP, (half + 1) * GRP)
                                    bc_ps = dps.tile([128, GRP], F32, tag="bcps",
                                                     name="bc_ps")
                                    nc.tensor.matmul(
                                        bc_ps[:], sel_a if half == 0 else sel_b,
                                        rc_sb[:], start=True, stop=True)
                                    nc.vector.tensor_copy(bc_sb[:, hsl], bc_ps[:])
                                nc.vector.tensor_mul(at_sb[h][:], au_sb[:],
                                                     bc_sb[:])
                            return [tail_ds, tail_bc]

                        deferred_tail[0] = make_tail(h, au_sb, dn0, dn1)
                    for fn in deferred_tail[0]:
                        fn()

                # ---------------- stage 3: o_proj partial ----------------
                with tc.tile_pool(name="ops", bufs=2, space="PSUM") as opps:
                    for st in range(8):
                        for half in range(2):
                            o_sb = ostpool.tile([128, 2048], BF16, tag="osb",
                                                name="o_sb")
                            o_ps = opps.tile([128, 2048], F32, tag="ops",
                                             name="o_ps")
                            for h in range(HPC):
                                for nn in range(4):
                                    n = half * 4 + nn
                                    nc.tensor.matmul(
                                        o_ps[:, nn * 512:(nn + 1) * 512],
                                        at_sb[h][:, st * 128:(st + 1) * 128],
                                        wo_sb[:, h * HID + n * 512:
                                              h * HID + (n + 1) * 512],
                                        start=(h == 0), stop=(h == HPC - 1),
                                    )
                            if (st + half) % 2 == 0:
                                nc.scalar.copy(o_sb[:], o_ps[:])
                            else:
                                nc.vector.tensor_copy(o_sb[:], o_ps[:])
                            nc.sync.dma_start(
                                outp[st * 128:(st + 1) * 128,
                                     half * 2048:(half + 1) * 2048],
                                o_sb[:],
                            )
    return nc


def _pack_ktiles(a, tile_rows=128):
    """[R, C] -> [128, (R//128)*C] with k-tile kt at cols [kt*C:(kt+1)*C]."""
    r, c = a.shape
    n = r // tile_rows
    return np.ascontiguousarray(
        a.reshape(n, tile_rows, c).transpose(1, 0, 2).reshape(tile_rows, n * c)
    )


def _rope_tables_ds(position_ids):
    """cos/sin in [d, s] layout: [128, Q] f64."""
    pos = np.asarray(position_ids).reshape(-1).astype(np.float64)      # [Q]
    inv_freq = 1.0 / (ROPE_THETA ** (np.arange(0, HD, 2, dtype=np.float64) / HD))
    ang_half = np.outer(inv_freq, pos)                                 # [64, Q]
    ang = np.concatenate([ang_half, ang_half], axis=0)                 # [128, Q]
    return np.cos(ang), np.sin(ang)


def kernel(hidden_states, attention_mask, position_ids, past_k, past_v,
           Wq, Wk, Wv, Wo):
    global LAST_RESULTS
    bf = ml_dtypes.bfloat16

    hs = np.asarray(hidden_states, np.float32).reshape(Q, HID)
    mask = np.asarray(attention_mask, np.float32).reshape(Q, KV)
    cos_d, sin_d = _rope_tables_ds(position_ids)

    scale = 1.0 / math.sqrt(HD)
    cosq_t = (cos_d * scale).astype(bf)
    sinq_t = (sin_d * scale).astype(bf)
    cosk_t = cos_d.astype(bf)
    sink_t = sin_d.astype(bf)

    # rotate-half permutation with sign: rot[d] = -x[d+64] (d<64); x[d-64]
    prot_np = np.zeros((128, 128), np.float32)
    for dd in range(64):
        prot_np[dd + 64, dd] = -1.0     # lhsT[d', d]: rot[d] += P[d', d] * x[d']
        prot_np[dd, dd + 64] = 1.0
    prot_t = prot_np.astype(bf)

    # diagonal masks: [128 kv, 4 tiles * 512 q]: kv tile 24+m vs queries
    # 0..511 (identical pattern to kv tile 28+m vs queries 512..1023)
    mask_t = np.empty((128, 2048), np.float32)
    for m in range(4):
        kt = 24 + m
        blk = mask[0:512, kt * 128:(kt + 1) * 128].T
        mask_t[:, m * 512:(m + 1) * 512] = (blk == 0.0).astype(np.float32)
    mask_t = mask_t.astype(bf)

    sel2_np = np.zeros((2, 256), np.float32)
    sel2_np[0, 0:128] = 1.0      # sel_a: broadcast rc row 0
    sel2_np[1, 128:256] = 1.0    # sel_b: broadcast rc row 1

    hst = _pack_ktiles(np.ascontiguousarray(hs.T)).astype(bf)      # [128, 32, 1024]
    hst = hst.reshape(128, N_HK, Q)

    nc = _build_program()
    in_maps = []
    for c in range(NCORES):
        qs = slice(c * HPC * HD, (c + 1) * HPC * HD)
        ks = slice(c * HD, (c + 1) * HD)
        wq_c = _pack_ktiles(
            np.ascontiguousarray(Wq[qs, :].T)
        ).astype(bf).reshape(128, N_HK, HPC * 128)
        wk_c = np.ascontiguousarray(Wk[ks, :].T)                   # [4096, 128]
        wv_c = np.ascontiguousarray(Wv[ks, :].T)
        wkv_c = _pack_ktiles(
            np.concatenate([wk_c, wv_c], axis=1)
        ).astype(bf).reshape(128, N_HK, 256)
        pkt = np.ascontiguousarray(past_k[0, c].T).astype(bf)      # [128, 3072]
        pv = _pack_ktiles(np.ascontiguousarray(past_v[0, c])).astype(bf)
        wo_c = _pack_ktiles(
            np.ascontiguousarray(Wo[:, qs].T)).astype(bf)          # [128, 4*4096]
        in_maps.append({
            "hst": hst, "wqt": wq_c, "wkvt": wkv_c, "pastkt": pkt,
            "pastv": pv, "maskt": mask_t, "cosq": cosq_t, "sinq": sinq_t,
            "cosk": cosk_t, "sink": sink_t, "prot": prot_t, "sel2": sel2_np,
            "wot": wo_c,
        })

    res = run_bass_kernel_spmd(nc, in_maps, list(range(NCORES)))
    LAST_RESULTS = res
    out = np.zeros((Q, HID), np.float32)
    for c in range(NCORES):
        out += np.asarray(res.results[c]["outp"], dtype=np.float32)
    return out.reshape(B, Q, HID)


# revision 6
# speedup vs baseline: 1.0762x; 1.0096x over previous
"""Llama GQA attention (B=1, Q=1024, PAST=3072, HID=4096, NH=32, NKV=8, HD=128)
tensor-parallel over heads across 8 NeuronCores.

Per core c: kv head c, query heads 4c..4c+3. Each core computes its partial
o_proj contribution [1024, 4096] in bf16; the host sums the 8 partials in f32.

v2 layout (vs v1): engineered to unload the DVE (v1 bottleneck: 441us busy).
  - q/k proj W-stationary: out is [d, seq] (born transposed, no PE transposes).
    RoPE rotate-half via a PE permutation matmul; combine = 3 DVE TT ops.
  - v proj hs-stationary: out is [seq, d] directly in attn lhsT layout.
  - scores land in bf16 PSUM supertiles [128, 2048] (2 kv tiles x 1024 q);
    exp is ONE fused ACTIVATE per supertile reading PSUM directly. No mask
    add except on the diagonal tiles (DVE, in-place in PSUM).
  - causal skip: kv tiles 28..31 only computed against queries 512..1023
    (one extra narrow-quad supertile); kv 24..27 masked only vs q 0..511,
    same [128,2048] mask pattern serves both diagonals.
  - softmax denom: DVE-primary / GpSimd (every 4th kv tile) accumulation,
    ones-matmul partition reduce accumulated in PSUM, reciprocal via
    reciprocal_approx_fast, broadcast via gpsimd partition_broadcast.
  - o_proj: PSUM->SBUF copies split Scalar/Vector, bf16 output partials.
"""

import math
import numpy as np
import ml_dtypes

import bass_rust
import concourse.bass as bass
import concourse.mybir as mybir
import concourse.tile as tile
from concourse.vector_clock import ScopedClock
from concourse.bass_utils import run_bass_kernel_spmd

# ---------------------------------------------------------------------------
# Workaround: walrus in this image rejects >1 sem wait on CTRL-class
# instructions (Drain/NoOp). TileContext's tail drain waits on every touched
# logical processor. Split the waits across preceding sync-engine nops.
MAX_WAITS = 1


def _split_waits(nc, inst):
    si = inst.ins.sync_info
    if si is None:
        return
    waits = list(si.on_wait)
    if len(waits) <= MAX_WAITS:
        return
    inst.ins.sync_info = bass_rust.SyncInfo(
        on_wait=waits[:MAX_WAITS], on_update=list(si.on_update)
    )
    rest = waits[MAX_WAITS:]
    while rest:
        extra = nc.sync.nop(nofuse=True)
        extra.ins.sync_info = bass_rust.SyncInfo(on_wait=rest[:MAX_WAITS], on_update=[])
        rest = rest[MAX_WAITS:]


def _drain_and_barrier_split(self, tick_clock, wait_clock):
    nc = self.nc
    carrier = nc.sync.nop(nofuse=True)
    wait_clock.add_sem_waits(carrier.ins, ScopedClock({None: tick_clock.global_clock}))
    _split_waits(nc, carrier)
    nc.sync.drain()
    nc.all_engine_barrier()
    popped = nc._tile_sem_poison_stack.pop()
    assert popped is self._sem_poison
    nc.clear_and_free_semaphores(list(self.sems.allocated().values()))
    nc.all_engine_barrier()


tile.TileContext._drain_and_barrier = _drain_and_barrier_split
# ---------------------------------------------------------------------------

# ---------------------------------------------------------------------------
# General wait-cap legalization: this walrus rejects instructions carrying
# more than a couple of sem waits. Post-process the BIR JSON: hoist overflow
# waits onto engine-matched NoOps inserted immediately before the offender
# (same engine queue -> same ordering semantics).
import json as _json

_CTRL_OPS = {"NoOp", "Drain", "EventSemaphore"}
_CAP_CTRL = 1
_CAP_OTHER = 1
_orig_to_json_bytes = bass.Bass.to_json_bytes


def _legalized_to_json_bytes(self, *a, **k):
    raw = _orig_to_json_bytes(self, *a, **k)
    m = _json.loads(raw)
    ctr = [0]
    changed = False
    for fn in m.get("functions", []):
        for blk in fn.get("blocks", []):
            insts = blk.get("instructions", [])
            out = []
            for ins in insts:
                si = ins.get("sync_info")
                if si:
                    waits = si.get("on_wait") or []
                    cap = _CAP_CTRL if ins.get("opcode") in _CTRL_OPS else _CAP_OTHER
                    if len(waits) > cap:
                        changed = True
                        rest = waits[:-cap]
                        si["on_wait"] = waits[-cap:]
                        while rest:
                            ctr[0] += 1
                            out.append({
                                "debug": ins.get("debug", 0),
                                "engine": ins["engine"],
                                "ins": [], "outs": [],
                                "name": f"{ins['name']}_lw{ctr[0]}",
                                "opcode": "NoOp",
                                "sync_info": {"on_wait": rest[:_CAP_CTRL],
                                              "on_update": []},
                            })
                            rest = rest[_CAP_CTRL:]
                out.append(ins)
            blk["instructions"] = out
    if not changed:
        return raw
    return _json.dumps(m).encode()


bass.Bass.to_json_bytes = _legalized_to_json_bytes
# ---------------------------------------------------------------------------


B, Q, PAST, HID = 1, 1024, 3072, 4096
NH, NKV, HD = 32, 8, 128
KV = PAST + Q           # 4096
NCORES = 8
HPC = NH // NCORES      # 4 query heads per core
ROPE_THETA = 10000.0
EXP_SHIFT = -20.0       # constant softmax shift (cancels exactly per row)

F32 = mybir.dt.float32
BF16 = mybir.dt.bfloat16

N_KT = KV // 128        # 32 kv tiles
N_HK = HID // 128       # 32 hid k-tiles
GRP = 512               # query group width (stage 1)
N_G = Q // GRP          # 2 groups
N_PV = PAST // 128      # 24 past-v tiles
N_SUP = 30              # stage-2 supertiles/head: 28 full-q kv tiles + 2 narrow pairs

LAST_RESULTS = None     # test harness reads exec_time_ns from here


def _build_program():
    nc = bass.Bass()
    hst = nc.declare_dram_parameter("hst", [128, N_HK, Q], BF16, isOutput=False)
    wqt = nc.declare_dram_parameter("wqt", [128, N_HK, HPC * 128], BF16, isOutput=False)
    wkvt = nc.declare_dram_parameter("wkvt", [128, N_HK, 256], BF16, isOutput=False)
    pastkt = nc.declare_dram_parameter("pastkt", [128, PAST], BF16, isOutput=False)
    pastv = nc.declare_dram_parameter("pastv", [128, PAST], BF16, isOutput=False)
    maskt = nc.declare_dram_parameter("maskt", [128, 2048], BF16, isOutput=False)
    # rope tables in [d, seq] layout; q tables pre-scaled by 1/sqrt(HD)
    cosq = nc.declare_dram_parameter("cosq", [128, Q], BF16, isOutput=False)
    sinq = nc.declare_dram_parameter("sinq", [128, Q], BF16, isOutput=False)
    cosk = nc.declare_dram_parameter("cosk", [128, Q], BF16, isOutput=False)
    sink = nc.declare_dram_parameter("sink", [128, Q], BF16, isOutput=False)
    prot = nc.declare_dram_parameter("prot", [128, 128], BF16, isOutput=False)
    sel2 = nc.declare_dram_parameter("sel2", [2, 256], BF16, isOutput=False)
    wot = nc.declare_dram_parameter("wot", [128, HPC * HID], BF16, isOutput=False)
    outp = nc.declare_dram_parameter("outp", [Q, HID], BF16, isOutput=True)

    with tile.TileContext(nc) as tc:
        with (
            tc.tile_pool(name="const", bufs=1) as cpool,
            tc.tile_pool(name="kvres", bufs=1) as kvpool,
            tc.tile_pool(name="qt", bufs=1) as qtpool,
            tc.tile_pool(name="attn", bufs=1) as apool,
        ):
            # ones2a/b: lhsT for denominator partition-reduce; row-select into
            # a shared [2, 512] PSUM bank (row 0 = q-half A, row 1 = q-half B)
            ones2a = cpool.tile([128, 2], BF16)
            nc.vector.memset(ones2a[:], 0.0)
            nc.vector.memset(ones2a[:, 0:1], 1.0)
            ones2b = cpool.tile([128, 2], BF16)
            nc.vector.memset(ones2b[:], 0.0)
            nc.vector.memset(ones2b[:, 1:2], 1.0)
            # sel_a/b: lhsT selecting row 0/1 of rc [2, 512] and broadcasting
            # it across all 128 output partitions (DMA'd: partition-sliced
            # memset is rejected by the BIR verifier)
            sel2_sb = cpool.tile([2, 256], BF16)
            sel_a = sel2_sb[:, 0:128]
            sel_b = sel2_sb[:, 128:256]
            shift_sb = cpool.tile([128, 1], F32)
            nc.vector.memset(shift_sb[:], EXP_SHIFT)
            prot_sb = cpool.tile([128, 128], BF16)
            mask_sb = cpool.tile([128, 2048], BF16)

            # K_T [128 d, KV] bf16; V packed [128 kv-sub, N_KT*128 d]
            kt_sb = kvpool.tile([128, KV], BF16)
            v_sb = kvpool.tile([128, N_KT * 128], BF16)

            # qT per head [128 d, Q] bf16; attnT per head [128 d, Q] bf16
            qt_sb = [qtpool.tile([128, Q], BF16, tag=f"qt{h}", name=f"qt{h}") for h in range(HPC)]
            at_sb = [apool.tile([128, Q], BF16, tag=f"at{h}", name=f"at{h}") for h in range(HPC)]

            # ---------------- stage 1: QKV projection + RoPE ----------------
            with (
                tc.tile_pool(name="hsw", bufs=1) as hspool,
                tc.tile_pool(name="rope", bufs=2) as rpool,
                tc.tile_pool(name="qkps", bufs=1, space="PSUM") as qkps,
                tc.tile_pool(name="vps", bufs=2, space="PSUM") as vps,
                tc.tile_pool(name="rotps", bufs=1, space="PSUM") as rotps,
            ):
                hs_sb = hspool.tile([128, N_HK, Q], BF16)
                wq_sb = hspool.tile([128, N_HK, HPC * 128], BF16)
                wkv_sb = hspool.tile([128, N_HK, 256], BF16)
                cosq_sb = hspool.tile([128, Q], BF16)
                sinq_sb = hspool.tile([128, Q], BF16)
                cosk_sb = hspool.tile([128, Q], BF16)
                sink_sb = hspool.tile([128, Q], BF16)
                # stage-1-critical loads first, chunked for early start
                for i in range(8):
                    s, e = i * (N_HK // 8), (i + 1) * (N_HK // 8)
                    nc.sync.dma_start(hs_sb[:, s:e, :], hst[:, s:e, :])
                    nc.sync.dma_start(wq_sb[:, s:e, :], wqt[:, s:e, :])
                    if i < 4:
                        s2, e2 = i * (N_HK // 4), (i + 1) * (N_HK // 4)
                        nc.sync.dma_start(wkv_sb[:, s2:e2, :], wkvt[:, s2:e2, :])
                nc.sync.dma_start(cosq_sb[:], cosq[:])
                nc.sync.dma_start(sinq_sb[:], sinq[:])
                nc.sync.dma_start(cosk_sb[:], cosk[:])
                nc.sync.dma_start(sink_sb[:], sink[:])
                nc.sync.dma_start(prot_sb[:], prot[:])
                nc.sync.dma_start(mask_sb[:], maskt[:])
                nc.sync.dma_start(sel2_sb[:], sel2[:])
                nc.sync.dma_start(kt_sb[:, :PAST], pastkt[:])
                nc.sync.dma_start(v_sb[:, : N_PV * 128], pastv[:])

                def rope(dst_bf, src_ps, cos_t, sin_t, g):
                    """dst_bf [128 d, 512 s] <- RoPE applied in [d, s] layout.

                    rot = P_rot.T @ src (PE permutation matmul, sign folded
                    into P_rot); dst = src*cos + rot*sin.
                    """
                    c = cos_t[:, g * GRP:(g + 1) * GRP]
                    s = sin_t[:, g * GRP:(g + 1) * GRP]
                    q_f = rpool.tile([128, GRP], BF16, tag="qf", name="q_f")
                    nc.scalar.copy(q_f[:], src_ps[:])
                    rot_ps = rotps.tile([128, GRP], F32, tag="rot", name="rot_ps")
                    nc.tensor.matmul(rot_ps[:], prot_sb[:], q_f[:],
                                     start=True, stop=True)
                    t1 = rpool.tile([128, GRP], F32, tag="t1", name="t1")
                    nc.vector.tensor_mul(t1[:], src_ps[:], c)
                    t2 = rpool.tile([128, GRP], F32, tag="t2", name="t2")
                    nc.vector.tensor_mul(t2[:], rot_ps[:], s)
                    nc.vector.tensor_add(dst_bf, t1[:], t2[:])

                for g in range(N_G):
                    gsl = slice(g * GRP, (g + 1) * GRP)
                    q_ps = [qkps.tile([128, GRP], F32, tag=f"qps{h}", name=f"qps{h}")
                            for h in range(HPC)]
                    k_ps = qkps.tile([128, GRP], F32, tag="kps", name="k_ps")
                    for k in range(N_HK):
                        rhs = hs_sb[:, k:k + 1, g * GRP:(g + 1) * GRP]
                        for h in range(HPC):
                            nc.tensor.matmul(
                                q_ps[h][:],
                                wq_sb[:, k:k + 1, h * 128:(h + 1) * 128],
                                rhs, start=(k == 0), stop=(k == N_HK - 1),
                            )
                        nc.tensor.matmul(
                            k_ps[:], wkv_sb[:, k:k + 1, 0:128], rhs,
                            start=(k == 0), stop=(k == N_HK - 1),
                        )
                    # rope first (q0 then k then q1-3) so stage 2 head 0
                    # can start while the v-loop still runs
                    rope(qt_sb[0][:, gsl], q_ps[0], cosq_sb, sinq_sb, g)
                    rope(kt_sb[:, PAST + g * GRP: PAST + (g + 1) * GRP],
                         k_ps, cosk_sb, sink_sb, g)
                    for h in range(1, HPC):
                        rope(qt_sb[h][:, gsl], q_ps[h], cosq_sb, sinq_sb, g)
                    # v proj: hs-stationary, out [seq, d] per 128-seq tile
                    for st in range(4):
                        gst = g * 4 + st
                        v_ps = vps.tile([128, 128], F32, tag="vp", name="v_ps")
                        for k in range(N_HK):
                            nc.tensor.matmul(
                                v_ps[:],
                                hs_sb[:, k:k + 1, gst * 128:(gst + 1) * 128],
                                wkv_sb[:, k:k + 1, 128:256],
                                start=(k == 0), stop=(k == N_HK - 1),
                            )
                        nc.scalar.copy(
                            v_sb[:, (N_PV + gst) * 128:(N_PV + gst + 1) * 128],
                            v_ps[:],
                        )

            # ------------- stage 2 + 3 (wo loads during stage 2) -------------
            with (
                tc.tile_pool(name="wo", bufs=1) as wopool,
                tc.tile_pool(name="ostage", bufs=2) as ostpool,
            ):
                wo_sb = wopool.tile([128, HPC * HID], BF16)
                for h in range(HPC):
                    nc.sync.dma_start(
                        wo_sb[:, h * HID:(h + 1) * HID],
                        wot[:, h * HID:(h + 1) * HID],
                    )

                # ---------------- stage 2: attention ----------------
                with (
                    tc.tile_pool(name="pt", bufs=6) as ptpool,
                    tc.tile_pool(name="softm", bufs=2) as smpool,
                    tc.tile_pool(name="scps", bufs=2, space="PSUM") as scps,
                    tc.tile_pool(name="aps", bufs=1, space="PSUM") as aps,
                    tc.tile_pool(name="dps", bufs=1, space="PSUM") as dps,
                ):
                    deferred_tail = [[]]

                    for h in range(HPC):
                        a_ps = aps.tile([128, Q], F32, tag="aacc", name="a_ps")
                        # denominator partials on DVE in bf16 (2 accumulators
                        # -> rounding error ~sqrt(15)*2^-9, well within budget)
                        dn0 = smpool.tile([128, Q], BF16, tag="dn0", name="dn0")
                        dn1 = smpool.tile([128, Q], BF16, tag="dn1", name="dn1")

                        def emit_attn(prev):
                            """Attn accumulation for the previous supertile
                            (lagged so the PE never waits on this supertile's
                            exp)."""
                            _, pt, plan = prev
                            for (kt, col, qoff) in plan:
                                nc.tensor.matmul(
                                    a_ps[:, qoff:qoff + GRP],
                                    v_sb[:, kt * 128:(kt + 1) * 128],
                                    pt[:, col:col + GRP],
                                    start=(kt == 24),
                                    stop=(kt == 23),
                                )

                        def dn_accum(jj, pt, plan):
                            if jj < 28:
                                key = jj % 2
                                dn = dn0 if key == 0 else dn1
                                if key not in touched:
                                    touched.add(key)
                                    nc.vector.tensor_copy(dn[:], pt[:])
                                else:
                                    nc.vector.tensor_add(dn[:], dn[:], pt[:])
                            else:
                                for (kt, col, qoff) in plan:
                                    dn = dn0 if kt % 2 == 0 else dn1
                                    nc.vector.tensor_add(
                                        dn[:, qoff:qoff + GRP],
                                        dn[:, qoff:qoff + GRP],
                                        pt[:, col:col + GRP],
                                    )

                        pend = []
                        touched = set()
                        # masked supertiles first, interleaved 1:1 with
                        # unmasked ones (gp mask-muls spread over ~13us);
                        # PSUM accumulation is order-independent. dn0/dn1
                        # first-touch stays on FULL supertiles (24, 25).
                        ORDER = ([24, 0, 25, 1, 26, 2, 27, 3] +
                                 list(range(4, 12)) + [28, 12, 29, 13] +
                                 list(range(14, 24)))
                        for pos, jj in enumerate(ORDER):
                            s_sup = scps.tile([128, 1024], F32, tag="ss", name="s_sup")
                            pt = ptpool.tile([128, 1024], BF16, tag="pt", name="pt")
                            plan = []
                            if jj < 28:
                                # kv tile jj x full q
                                kt = jj
                                for ii in range(2):
                                    nc.tensor.matmul(
                                        s_sup[:, ii * GRP:(ii + 1) * GRP],
                                        kt_sb[:, kt * 128:(kt + 1) * 128],
                                        qt_sb[h][:, ii * GRP:(ii + 1) * GRP],
                                        start=True, stop=True,
                                    )
                                    plan.append((kt, ii * GRP, ii * GRP))
                            else:
                                # narrow pair: kv (28,29) or (30,31) x q-half B
                                for c in range(2):
                                    kt = 28 + 2 * (jj - 28) + c
                                    nc.tensor.matmul(
                                        s_sup[:, c * GRP:(c + 1) * GRP],
                                        kt_sb[:, kt * 128:(kt + 1) * 128],
                                        qt_sb[h][:, GRP:Q],
                                        start=True, stop=True,
                                    )
                                    plan.append((kt, c * GRP, GRP))
                            nc.scalar.activation(
                                pt[:], s_sup[:],
                                mybir.ActivationFunctionType.Exp,
                                bias=shift_sb[:], scale=1.0,
                            )
                            if jj >= 24:
                                if jj < 28:
                                    nc.gpsimd.tensor_mul(
                                        pt[:, 0:GRP], pt[:, 0:GRP],
                                        mask_sb[:, (jj - 24) * GRP:(jj - 23) * GRP],
                                    )
                                else:
                                    for (kt, col, qoff) in plan:
                                        nc.gpsimd.tensor_mul(
                                            pt[:, col:col + GRP],
                                            pt[:, col:col + GRP],
                                            mask_sb[:, (kt - 28) * GRP:(kt - 27) * GRP],
                                        )
                            dn_accum(jj, pt, plan)
                            pend.append((jj, pt, plan))
                            if len(pend) > 2:
                                emit_attn(pend.pop(0))
                            if pos in (2, 8) and deferred_tail[0]:
                                deferred_tail[0].pop(0)()
                        for ent in pend:
                            emit_attn(ent)
                        # copy a_ps out unnormalized right away (frees a_ps
                        # for the next head); the whole denominator/normalize
                        # tail is deferred into the next head's loop so its
                        # PE ops never block the next head's scores while
                        # waiting on the DVE dn-drain/reciprocal
                        au_sb = smpool.tile([128, Q], BF16, tag="atu", name="au_sb")
                        nc.vector.tensor_copy(au_sb[:], a_ps[:])

                        def make_tail(h, au_sb, dn0, dn1):
                            rc_sb = smpool.tile([2, GRP], BF16, tag="recip",
                                                name="rc_sb")

                            def tail_ds():
                                ds_ps = dps.tile([2, GRP], F32, tag="dsum",
                                                 name="ds_ps")
                                for idx, (sel, dn, hoff) in enumerate(
                                    [(ones2a, dn0, 0), (ones2a, dn1, 0),
                                     (ones2b, dn0, GRP), (ones2b, dn1, GRP)]
                                ):
                                    nc.tensor.matmul(
                                        ds_ps[:], sel[:], dn[:, hoff:hoff + GRP],
                                        start=(idx == 0), stop=(idx == 3),
                                    )
                                with nc.allow_low_precision(
                                        reason="1/denom to bf16: 0.4% rounding"):
                                    nc.vector.reciprocal(rc_sb[:], ds_ps[:])

                            def tail_bc():
                                bc_sb = smpool.tile([128, Q], F32, tag="bcast",
                                                    name="bc_sb")
                                for half in range(2):
                                    hsl = slice(half * GRP, (half + 1) * GRP)
                                    bc_ps = dps.tile([128, GRP], F32, tag="bcps",
                                                     name="bc_ps")
                                    nc.tensor.matmul(
                                        bc_ps[:], sel_a if half == 0 else sel_b,
                                        rc_sb[:], start=True, stop=True)
                                    nc.vector.tensor_copy(bc_sb[:, hsl], bc_ps[:])
                                nc.vector.tensor_mul(at_sb[h][:], au_sb[:],
                                                     bc_sb[:])
                            return [tail_ds, tail_bc]

                        deferred_tail[0] = make_tail(h, au_sb, dn0, dn1)
                    for fn in deferred_tail[0]:
                        fn()

                # ---------------- stage 3: o_proj partial ----------------
                with tc.tile_pool(name="ops", bufs=2, space="PSUM") as opps:
                    for st in range(8):
                        for half in range(2):
                            o_sb = ostpool.tile([128, 2048], BF16, tag="osb",
                                                name="o_sb")
                            o_ps = opps.tile([128, 2048], F32, tag="ops",
                                             name="o_ps")
                            for h in range(HPC):
                                for nn in range(4):
                                    n = half * 4 + nn
                                    nc.tensor.matmul(
                                        o_ps[:, nn * 512:(nn + 1) * 512],
                                        at_sb[h][:, st * 128:(st + 1) * 128],
                                        wo_sb[:, h * HID + n * 512:
                                              h * HID + (n + 1) * 512],
                                        start=(h == 0), stop=(h == HPC - 1),
                                    )
                            if (st + half) % 2 == 0:
                                nc.scalar.copy(o_sb[:], o_ps[:])
                            else:
                                nc.vector.tensor_copy(o_sb[:], o_ps[:])
                            nc.sync.dma_start(
                                outp[st * 128:(st + 1) * 128,
                                     half * 2048:(half + 1) * 2048],
                                o_sb[:],
                            )
    return nc


def _pack_ktiles(a, tile_rows=128):
    """[R, C] -> [128, (R//128)*C] with k-tile kt at cols [kt*C:(kt+1)*C]."""
    r, c = a.shape
    n = r // tile_rows
    return np.ascontiguousarray(
        a.reshape(n, tile_rows, c).transpose(1, 0, 2).reshape(tile_rows, n * c)
    )


def _rope_tables_ds(position_ids):
    """cos/sin in [d, s] layout: [128, Q] f64."""
    pos = np.asarray(position_ids).reshape(-1).astype(np.float64)      # [Q]
    inv_freq = 1.0 / (ROPE_THETA ** (np.arange(0, HD, 2, dtype=np.float64) / HD))
    ang_half = np.outer(inv_freq, pos)                                 # [64, Q]
    ang = np.concatenate([ang_half, ang_half], axis=0)                 # [128, Q]
    return np.cos(ang), np.sin(ang)


def kernel(hidden_states, attention_mask, position_ids, past_k, past_v,
           Wq, Wk, Wv, Wo):
    global LAST_RESULTS
    bf = ml_dtypes.bfloat16

    hs = np.asarray(hidden_states, np.float32).reshape(Q, HID)
    mask = np.asarray(attention_mask, np.float32).reshape(Q, KV)
    cos_d, sin_d = _rope_tables_ds(position_ids)

    scale = 1.0 / math.sqrt(HD)
    cosq_t = (cos_d * scale).astype(bf)
    sinq_t = (sin_d * scale).astype(bf)
    cosk_t = cos_d.astype(bf)
    sink_t = sin_d.astype(bf)

    # rotate-half permutation with sign: rot[d] = -x[d+64] (d<64); x[d-64]
    prot_np = np.zeros((128, 128), np.float32)
    for dd in range(64):
        prot_np[dd + 64, dd] = -1.0     # lhsT[d', d]: rot[d] += P[d', d] * x[d']
        prot_np[dd, dd + 64] = 1.0
    prot_t = prot_np.astype(bf)

    # diagonal masks: [128 kv, 4 tiles * 512 q]: kv tile 24+m vs queries
    # 0..511 (identical pattern to kv tile 28+m vs queries 512..1023)
    mask_t = np.empty((128, 2048), np.float32)
    for m in range(4):
        kt = 24 + m
        blk = mask[0:512, kt * 128:(kt + 1) * 128].T
        mask_t[:, m * 512:(m + 1) * 512] = (blk == 0.0).astype(np.float32)
    mask_t = mask_t.astype(bf)

    sel2_np = np.zeros((2, 256), np.float32)  # cast to bf16 below
    sel2_np[0, 0:128] = 1.0      # sel_a: broadcast rc row 0
    sel2_np[1, 128:256] = 1.0    # sel_b: broadcast rc row 1

    hst = _pack_ktiles(np.ascontiguousarray(hs.T)).astype(bf)      # [128, 32, 1024]
    hst = hst.reshape(128, N_HK, Q)

    nc = _build_program()
    in_maps = []
    for c in range(NCORES):
        qs = slice(c * HPC * HD, (c + 1) * HPC * HD)
        ks = slice(c * HD, (c + 1) * HD)
        wq_c = _pack_ktiles(
            np.ascontiguousarray(Wq[qs, :].T)
        ).astype(bf).reshape(128, N_HK, HPC * 128)
        wk_c = np.ascontiguousarray(Wk[ks, :].T)                   # [4096, 128]
        wv_c = np.ascontiguousarray(Wv[ks, :].T)
        wkv_c = _pack_ktiles(
            np.concatenate([wk_c, wv_c], axis=1)
        ).astype(bf).reshape(128, N_HK, 256)
        pkt = np.ascontiguousarray(past_k[0, c].T).astype(bf)      # [128, 3072]
        pv = _pack_ktiles(np.ascontiguousarray(past_v[0, c])).astype(bf)
        wo_c = _pack_ktiles(
            np.ascontiguousarray(Wo[:, qs].T)).astype(bf)          # [128, 4*4096]
        in_maps.append({
            "hst": hst, "wqt": wq_c, "wkvt": wkv_c, "pastkt": pkt,
            "pastv": pv, "maskt": mask_t, "cosq": cosq_t, "sinq": sinq_t,
            "cosk": cosk_t, "sink": sink_t, "prot": prot_t,
            "sel2": sel2_np.astype(bf),
            "wot": wo_c,
        })

    res = run_bass_kernel_spmd(nc, in_maps, list(range(NCORES)))
    LAST_RESULTS = res
    out = np.zeros((Q, HID), np.float32)
    for c in range(NCORES):
        out += np.asarray(res.results[c]["outp"], dtype=np.float32)
    return out.reshape(B, Q, HID)


# revision 7
# speedup vs baseline: 1.0927x; 1.0153x over previous
"""Llama GQA attention (B=1, Q=1024, PAST=3072, HID=4096, NH=32, NKV=8, HD=128)
tensor-parallel over heads across 8 NeuronCores.

Per core c: kv head c, query heads 4c..4c+3. Each core computes its partial
o_proj contribution [1024, 4096] in bf16; the host sums the 8 partials in f32.

v2 layout (vs v1): engineered to unload the DVE (v1 bottleneck: 441us busy).
  - q/k proj W-stationary: out is [d, seq] (born transposed, no PE transposes).
    RoPE rotate-half via a PE permutation matmul; combine = 3 DVE TT ops.
  - v proj hs-stationary: out is [seq, d] directly in attn lhsT layout.
  - scores land in bf16 PSUM supertiles [128, 2048] (2 kv tiles x 1024 q);
    exp is ONE fused ACTIVATE per supertile reading PSUM directly. No mask
    add except on the diagonal tiles (DVE, in-place in PSUM).
  - causal skip: kv tiles 28..31 only computed against queries 512..1023
    (one extra narrow-quad supertile); kv 24..27 masked only vs q 0..511,
    same [128,2048] mask pattern serves both diagonals.
  - softmax denom: DVE-primary / GpSimd (every 4th kv tile) accumulation,
    ones-matmul partition reduce accumulated in PSUM, reciprocal via
    reciprocal_approx_fast, broadcast via gpsimd partition_broadcast.
  - o_proj: PSUM->SBUF copies split Scalar/Vector, bf16 output partials.
"""

import math
import numpy as np
import ml_dtypes

import bass_rust
import concourse.bass as bass
import concourse.mybir as mybir
import concourse.tile as tile
from concourse.vector_clock import ScopedClock
from concourse.bass_utils import run_bass_kernel_spmd

# ---------------------------------------------------------------------------
# Workaround: walrus in this image rejects >1 sem wait on CTRL-class
# instructions (Drain/NoOp). TileContext's tail drain waits on every touched
# logical processor. Split the waits across preceding sync-engine nops.
MAX_WAITS = 1


def _split_waits(nc, inst):
    si = inst.ins.sync_info
    if si is None:
        return
    waits = list(si.on_wait)
    if len(waits) <= MAX_WAITS:
        return
    inst.ins.sync_info = bass_rust.SyncInfo(
        on_wait=waits[:MAX_WAITS], on_update=list(si.on_update)
    )
    rest = waits[MAX_WAITS:]
    while rest:
        extra = nc.sync.nop(nofuse=True)
        extra.ins.sync_info = bass_rust.SyncInfo(on_wait=rest[:MAX_WAITS], on_update=[])
        rest = rest[MAX_WAITS:]


def _drain_and_barrier_split(self, tick_clock, wait_clock):
    nc = self.nc
    carrier = nc.sync.nop(nofuse=True)
    wait_clock.add_sem_waits(carrier.ins, ScopedClock({None: tick_clock.global_clock}))
    _split_waits(nc, carrier)
    nc.sync.drain()
    nc.all_engine_barrier()
    popped = nc._tile_sem_poison_stack.pop()
    assert popped is self._sem_poison
    nc.clear_and_free_semaphores(list(self.sems.allocated().values()))
    nc.all_engine_barrier()


tile.TileContext._drain_and_barrier = _drain_and_barrier_split
# ---------------------------------------------------------------------------

# ---------------------------------------------------------------------------
# General wait-cap legalization: this walrus rejects instructions carrying
# more than a couple of sem waits. Post-process the BIR JSON: hoist overflow
# waits onto engine-matched NoOps inserted immediately before the offender
# (same engine queue -> same ordering semantics).
import json as _json

_CTRL_OPS = {"NoOp", "Drain", "EventSemaphore"}
_CAP_CTRL = 1
_CAP_OTHER = 1
_orig_to_json_bytes = bass.Bass.to_json_bytes


def _legalized_to_json_bytes(self, *a, **k):
    raw = _orig_to_json_bytes(self, *a, **k)
    m = _json.loads(raw)
    ctr = [0]
    changed = False
    for fn in m.get("functions", []):
        for blk in fn.get("blocks", []):
            insts = blk.get("instructions", [])
            out = []
            for ins in insts:
                si = ins.get("sync_info")
                if si:
                    waits = si.get("on_wait") or []
                    cap = _CAP_CTRL if ins.get("opcode") in _CTRL_OPS else _CAP_OTHER
                    if len(waits) > cap:
                        changed = True
                        rest = waits[:-cap]
                        si["on_wait"] = waits[-cap:]
                        while rest:
                            ctr[0] += 1
                            out.append({
                                "debug": ins.get("debug", 0),
                                "engine": ins["engine"],
                                "ins": [], "outs": [],
                                "name": f"{ins['name']}_lw{ctr[0]}",
                                "opcode": "NoOp",
                                "sync_info": {"on_wait": rest[:_CAP_CTRL],
                                              "on_update": []},
                            })
                            rest = rest[_CAP_CTRL:]
                out.append(ins)
            blk["instructions"] = out
    if not changed:
        return raw
    return _json.dumps(m).encode()


bass.Bass.to_json_bytes = _legalized_to_json_bytes
# ---------------------------------------------------------------------------


B, Q, PAST, HID = 1, 1024, 3072, 4096
NH, NKV, HD = 32, 8, 128
KV = PAST + Q           # 4096
NCORES = 8
HPC = NH // NCORES      # 4 query heads per core
ROPE_THETA = 10000.0
EXP_SHIFT = -20.0       # constant softmax shift (cancels exactly per row)

F32 = mybir.dt.float32
BF16 = mybir.dt.bfloat16

N_KT = KV // 128        # 32 kv tiles
N_HK = HID // 128       # 32 hid k-tiles
GRP = 512               # query group width (stage 1)
N_G = Q // GRP          # 2 groups
N_PV = PAST // 128      # 24 past-v tiles
N_SUP = 30              # stage-2 supertiles/head: 28 full-q kv tiles + 2 narrow pairs

LAST_RESULTS = None     # test harness reads exec_time_ns from here


def _build_program():
    nc = bass.Bass()
    hst = nc.declare_dram_parameter("hst", [128, N_HK, Q], BF16, isOutput=False)
    wqt = nc.declare_dram_parameter("wqt", [128, N_HK, HPC * 128], BF16, isOutput=False)
    wkvt = nc.declare_dram_parameter("wkvt", [128, N_HK, 256], BF16, isOutput=False)
    pastkt = nc.declare_dram_parameter("pastkt", [128, PAST], BF16, isOutput=False)
    pastv = nc.declare_dram_parameter("pastv", [128, PAST], BF16, isOutput=False)
    maskt = nc.declare_dram_parameter("maskt", [128, 2048], BF16, isOutput=False)
    # rope tables in [d, seq] layout; q tables pre-scaled by 1/sqrt(HD)
    cosq = nc.declare_dram_parameter("cosq", [128, Q], BF16, isOutput=False)
    sinq = nc.declare_dram_parameter("sinq", [128, Q], BF16, isOutput=False)
    cosk = nc.declare_dram_parameter("cosk", [128, Q], BF16, isOutput=False)
    sink = nc.declare_dram_parameter("sink", [128, Q], BF16, isOutput=False)
    prot = nc.declare_dram_parameter("prot", [128, 128], BF16, isOutput=False)
    sel2 = nc.declare_dram_parameter("sel2", [2, 256], BF16, isOutput=False)
    wot = nc.declare_dram_parameter("wot", [128, HPC * HID], BF16, isOutput=False)
    outp = nc.declare_dram_parameter("outp", [Q, HID], BF16, isOutput=True)

    with tile.TileContext(nc) as tc:
        with (
            tc.tile_pool(name="const", bufs=1) as cpool,
            tc.tile_pool(name="kvres", bufs=1) as kvpool,
            tc.tile_pool(name="qt", bufs=1) as qtpool,
            tc.tile_pool(name="attn", bufs=1) as apool,
        ):
            # ones2a/b: lhsT for denominator partition-reduce; row-select into
            # a shared [2, 512] PSUM bank (row 0 = q-half A, row 1 = q-half B)
            ones2a = cpool.tile([128, 2], BF16)
            nc.vector.memset(ones2a[:], 0.0)
            nc.vector.memset(ones2a[:, 0:1], 1.0)
            ones2b = cpool.tile([128, 2], BF16)
            nc.vector.memset(ones2b[:], 0.0)
            nc.vector.memset(ones2b[:, 1:2], 1.0)
            # sel_a/b: lhsT selecting row 0/1 of rc [2, 512] and broadcasting
            # it across all 128 output partitions (DMA'd: partition-sliced
            # memset is rejected by the BIR verifier)
            sel2_sb = cpool.tile([2, 256], BF16)
            sel_a = sel2_sb[:, 0:128]
            sel_b = sel2_sb[:, 128:256]
            shift_sb = cpool.tile([128, 1], F32)
            nc.vector.memset(shift_sb[:], EXP_SHIFT)
            prot_sb = cpool.tile([128, 128], BF16)
            mask_sb = cpool.tile([128, 2048], BF16)

            # K_T [128 d, KV] bf16; V packed [128 kv-sub, N_KT*128 d]
            kt_sb = kvpool.tile([128, KV], BF16)
            v_sb = kvpool.tile([128, N_KT * 128], BF16)

            # qT per head [128 d, Q] bf16; attnT per head [128 d, Q] bf16
            qt_sb = [qtpool.tile([128, Q], BF16, tag=f"qt{h}", name=f"qt{h}") for h in range(HPC)]
            at_sb = [apool.tile([128, Q], BF16, tag=f"at{h}", name=f"at{h}") for h in range(HPC)]

            # ---------------- stage 1: QKV projection + RoPE ----------------
            with (
                tc.tile_pool(name="hsw", bufs=1) as hspool,
                tc.tile_pool(name="rope", bufs=2) as rpool,
                tc.tile_pool(name="qkps", bufs=1, space="PSUM") as qkps,
                tc.tile_pool(name="vps", bufs=2, space="PSUM") as vps,
                tc.tile_pool(name="rotps", bufs=1, space="PSUM") as rotps,
            ):
                hs_sb = hspool.tile([128, N_HK, Q], BF16)
                wq_sb = hspool.tile([128, N_HK, HPC * 128], BF16)
                wkv_sb = hspool.tile([128, N_HK, 256], BF16)
                cosq_sb = hspool.tile([128, Q], BF16)
                sinq_sb = hspool.tile([128, Q], BF16)
                cosk_sb = hspool.tile([128, Q], BF16)
                sink_sb = hspool.tile([128, Q], BF16)
                # stage-1-critical loads first; finer leading chunks so the
                # first q/k matmuls start ~2.5us in instead of ~10us
                bounds = [0, 2, 4, 6, 8, 12, 16, 20, 24, 28, 32]
                for i in range(len(bounds) - 1):
                    s, e = bounds[i], bounds[i + 1]
                    nc.sync.dma_start(hs_sb[:, s:e, :], hst[:, s:e, :])
                    nc.sync.dma_start(wq_sb[:, s:e, :], wqt[:, s:e, :])
                    nc.sync.dma_start(wkv_sb[:, s:e, :], wkvt[:, s:e, :])
                nc.sync.dma_start(cosq_sb[:], cosq[:])
                nc.sync.dma_start(sinq_sb[:], sinq[:])
                nc.sync.dma_start(cosk_sb[:], cosk[:])
                nc.sync.dma_start(sink_sb[:], sink[:])
                nc.sync.dma_start(prot_sb[:], prot[:])
                nc.sync.dma_start(mask_sb[:], maskt[:])
                nc.sync.dma_start(sel2_sb[:], sel2[:])
                nc.sync.dma_start(kt_sb[:, :PAST], pastkt[:])
                nc.sync.dma_start(v_sb[:, : N_PV * 128], pastv[:])

                def rope(dst_bf, src_ps, cos_t, sin_t, g):
                    """dst_bf [128 d, 512 s] <- RoPE applied in [d, s] layout.

                    rot = P_rot.T @ src (PE permutation matmul, sign folded
                    into P_rot); dst = src*cos + rot*sin.
                    """
                    c = cos_t[:, g * GRP:(g + 1) * GRP]
                    s = sin_t[:, g * GRP:(g + 1) * GRP]
                    q_f = rpool.tile([128, GRP], BF16, tag="qf", name="q_f")
                    nc.scalar.copy(q_f[:], src_ps[:])
                    rot_ps = rotps.tile([128, GRP], F32, tag="rot", name="rot_ps")
                    nc.tensor.matmul(rot_ps[:], prot_sb[:], q_f[:],
                                     start=True, stop=True)
                    t1 = rpool.tile([128, GRP], F32, tag="t1", name="t1")
                    nc.vector.tensor_mul(t1[:], src_ps[:], c)
                    t2 = rpool.tile([128, GRP], F32, tag="t2", name="t2")
                    nc.vector.tensor_mul(t2[:], rot_ps[:], s)
                    nc.vector.tensor_add(dst_bf, t1[:], t2[:])

                for g in range(N_G):
                    gsl = slice(g * GRP, (g + 1) * GRP)
                    q_ps = [qkps.tile([128, GRP], F32, tag=f"qps{h}", name=f"qps{h}")
                            for h in range(HPC)]
                    k_ps = qkps.tile([128, GRP], F32, tag="kps", name="k_ps")
                    for k in range(N_HK):
                        rhs = hs_sb[:, k:k + 1, g * GRP:(g + 1) * GRP]
                        for h in range(HPC):
                            nc.tensor.matmul(
                                q_ps[h][:],
                                wq_sb[:, k:k + 1, h * 128:(h + 1) * 128],
                                rhs, start=(k == 0), stop=(k == N_HK - 1),
                            )
                        nc.tensor.matmul(
                            k_ps[:], wkv_sb[:, k:k + 1, 0:128], rhs,
                            start=(k == 0), stop=(k == N_HK - 1),
                        )
                    # rope first (q0 then k then q1-3) so stage 2 head 0
                    # can start while the v-loop still runs
                    rope(qt_sb[0][:, gsl], q_ps[0], cosq_sb, sinq_sb, g)
                    rope(kt_sb[:, PAST + g * GRP: PAST + (g + 1) * GRP],
                         k_ps, cosk_sb, sink_sb, g)
                    for h in range(1, HPC):
                        rope(qt_sb[h][:, gsl], q_ps[h], cosq_sb, sinq_sb, g)
                    # v proj: hs-stationary, out [seq, d] per 128-seq tile
                    for st in range(4):
                        gst = g * 4 + st
                        v_ps = vps.tile([128, 128], F32, tag="vp", name="v_ps")
                        for k in range(N_HK):
                            nc.tensor.matmul(
                                v_ps[:],
                                hs_sb[:, k:k + 1, gst * 128:(gst + 1) * 128],
                                wkv_sb[:, k:k + 1, 128:256],
                                start=(k == 0), stop=(k == N_HK - 1),
                            )
                        nc.scalar.copy(
                            v_sb[:, (N_PV + gst) * 128:(N_PV + gst + 1) * 128],
                            v_ps[:],
                        )

            # ------------- stage 2 + 3 (wo loads during stage 2) -------------
            with (
                tc.tile_pool(name="wo", bufs=1) as wopool,
                tc.tile_pool(name="ostage", bufs=2) as ostpool,
            ):
                wo_sb = wopool.tile([128, HPC * HID], BF16)
                for h in range(HPC):
                    nc.sync.dma_start(
                        wo_sb[:, h * HID:(h + 1) * HID],
                        wot[:, h * HID:(h + 1) * HID],
                    )

                # ---------------- stage 2: attention ----------------
                with (
                    tc.tile_pool(name="pt", bufs=6) as ptpool,
                    tc.tile_pool(name="softm", bufs=2) as smpool,
                    tc.tile_pool(name="scps", bufs=2, space="PSUM") as scps,
                    tc.tile_pool(name="aps", bufs=1, space="PSUM") as aps,
                    tc.tile_pool(name="dps", bufs=1, space="PSUM") as dps,
                ):
                    deferred_tail = [[]]

                    for h in range(HPC):
                        a_ps = aps.tile([128, Q], F32, tag="aacc", name="a_ps")
                        # denominator partials on DVE in bf16 (2 accumulators
                        # -> rounding error ~sqrt(15)*2^-9, well within budget)
                        dn0 = smpool.tile([128, Q], BF16, tag="dn0", name="dn0")
                        dn1 = smpool.tile([128, Q], BF16, tag="dn1", name="dn1")

                        def emit_attn(prev):
                            """Attn accumulation for the previous supertile
                            (lagged so the PE never waits on this supertile's
                            exp)."""
                            _, pt, plan = prev
                            for (kt, col, qoff) in plan:
                                nc.tensor.matmul(
                                    a_ps[:, qoff:qoff + GRP],
                                    v_sb[:, kt * 128:(kt + 1) * 128],
                                    pt[:, col:col + GRP],
                                    start=(kt == 24),
                                    stop=(kt == 23),
                                )

                        def dn_accum(jj, pt, plan):
                            if jj < 28:
                                key = jj % 2
                                dn = dn0 if key == 0 else dn1
                                if key not in touched:
                                    touched.add(key)
                                    nc.vector.tensor_copy(dn[:], pt[:])
                                else:
                                    nc.vector.tensor_add(dn[:], dn[:], pt[:])
                            else:
                                for (kt, col, qoff) in plan:
                                    dn = dn0 if kt % 2 == 0 else dn1
                                    nc.vector.tensor_add(
                                        dn[:, qoff:qoff + GRP],
                                        dn[:, qoff:qoff + GRP],
                                        pt[:, col:col + GRP],
                                    )

                        pend = []
                        touched = set()
                        # masked supertiles first, interleaved 1:1 with
                        # unmasked ones (gp mask-muls spread over ~13us);
                        # PSUM accumulation is order-independent. dn0/dn1
                        # first-touch stays on FULL supertiles (24, 25).
                        ORDER = ([24, 0, 25, 1, 26, 2, 27, 3] +
                                 list(range(4, 12)) + [28, 12, 29, 13] +
                                 list(range(14, 24)))
                        for pos, jj in enumerate(ORDER):
                            s_sup = scps.tile([128, 1024], F32, tag="ss", name="s_sup")
                            pt = ptpool.tile([128, 1024], BF16, tag="pt", name="pt")
                            plan = []
                            if jj < 28:
                                # kv tile jj x full q
                                kt = jj
                                for ii in range(2):
                                    nc.tensor.matmul(
                                        s_sup[:, ii * GRP:(ii + 1) * GRP],
                                        kt_sb[:, kt * 128:(kt + 1) * 128],
                                        qt_sb[h][:, ii * GRP:(ii + 1) * GRP],
                                        start=True, stop=True,
                                    )
                                    plan.append((kt, ii * GRP, ii * GRP))
                            else:
                                # narrow pair: kv (28,29) or (30,31) x q-half B
                                for c in range(2):
                                    kt = 28 + 2 * (jj - 28) + c
                                    nc.tensor.matmul(
                                        s_sup[:, c * GRP:(c + 1) * GRP],
                                        kt_sb[:, kt * 128:(kt + 1) * 128],
                                        qt_sb[h][:, GRP:Q],
                                        start=True, stop=True,
                                    )
                                    plan.append((kt, c * GRP, GRP))
                            nc.scalar.activation(
                                pt[:], s_sup[:],
                                mybir.ActivationFunctionType.Exp,
                                bias=shift_sb[:], scale=1.0,
                            )
                            if jj >= 24:
                                if jj < 28:
                                    nc.gpsimd.tensor_mul(
                                        pt[:, 0:GRP], pt[:, 0:GRP],
                                        mask_sb[:, (jj - 24) * GRP:(jj - 23) * GRP],
                                    )
                                else:
                                    for (kt, col, qoff) in plan:
                                        nc.gpsimd.tensor_mul(
                                            pt[:, col:col + GRP],
                                            pt[:, col:col + GRP],
                                            mask_sb[:, (kt - 28) * GRP:(kt - 27) * GRP],
                                        )
                            dn_accum(jj, pt, plan)
                            pend.append((jj, pt, plan))
                            if len(pend) > 2:
                                emit_attn(pend.pop(0))
                            if pos in (2, 8) and deferred_tail[0]:
                                deferred_tail[0].pop(0)()
                        for ent in pend:
                            emit_attn(ent)
                        # copy a_ps out unnormalized right away (frees a_ps
                        # for the next head); the whole denominator/normalize
                        # tail is deferred into the next head's loop so its
                        # PE ops never block the next head's scores while
                        # waiting on the DVE dn-drain/reciprocal
                        au_sb = smpool.tile([128, Q], BF16, tag="atu", name="au_sb")
                        nc.vector.tensor_copy(au_sb[:, 0:GRP], a_ps[:, 0:GRP])
                        nc.scalar.copy(au_sb[:, GRP:Q], a_ps[:, GRP:Q])

                        def make_tail(h, au_sb, dn0, dn1):
                            rc_sb = smpool.tile([2, GRP], BF16, tag="recip",
                                                name="rc_sb")

                            def tail_ds():
                                ds_ps = dps.tile([2, GRP], F32, tag="dsum",
                                                 name="ds_ps")
                                for idx, (sel, dn, hoff) in enumerate(
                                    [(ones2a, dn0, 0), (ones2a, dn1, 0),
                                     (ones2b, dn0, GRP), (ones2b, dn1, GRP)]
                                ):
                                    nc.tensor.matmul(
                                        ds_ps[:], sel[:], dn[:, hoff:hoff + GRP],
                                        start=(idx == 0), stop=(idx == 3),
                                    )
                                with nc.allow_low_precision(
                                        reason="1/denom to bf16: 0.4% rounding"):
                                    nc.vector.reciprocal(rc_sb[:], ds_ps[:])

                            def tail_bc():
                                bc_sb = smpool.tile([128, Q], F32, tag="bcast",
                                                    name="bc_sb")
                                for half in range(2):
                                    hsl = slice(half * GRP, (half + 1) * GRP)
                                    bc_ps = dps.tile([128, GRP], F32, tag="bcps",
                                                     name="bc_ps")
                                    nc.tensor.matmul(
                                        bc_ps[:], sel_a if half == 0 else sel_b,
                                        rc_sb[:], start=True, stop=True)
                                    nc.vector.tensor_copy(bc_sb[:, hsl], bc_ps[:])
                                nc.vector.tensor_mul(at_sb[h][:], au_sb[:],
                                                     bc_sb[:])
                            return [tail_ds, tail_bc]

                        deferred_tail[0] = make_tail(h, au_sb, dn0, dn1)
                    for fn in deferred_tail[0]:
                        fn()

                # ---------------- stage 3: o_proj partial ----------------
                with tc.tile_pool(name="ops", bufs=2, space="PSUM") as opps:
                    for st in range(8):
                        for half in range(2):
                            o_sb = ostpool.tile([128, 2048], BF16, tag="osb",
                                                name="o_sb")
                            o_ps = opps.tile([128, 2048], F32, tag="ops",
                                             name="o_ps")
                            for h in range(HPC):
                                for nn in range(4):
                                    n = half * 4 + nn
                                    nc.tensor.matmul(
                                        o_ps[:, nn * 512:(nn + 1) * 512],
                                        at_sb[h][:, st * 128:(st + 1) * 128],
                                        wo_sb[:, h * HID + n * 512:
                                              h * HID + (n + 1) * 512],
                                        start=(h == 0), stop=(h == HPC - 1),
                                    )
                            if (st + half) % 2 == 0:
                                nc.scalar.copy(o_sb[:], o_ps[:])
                            else:
                                nc.vector.tensor_copy(o_sb[:], o_ps[:])
                            nc.sync.dma_start(
                                outp[st * 128:(st + 1) * 128,
                                     half * 2048:(half + 1) * 2048],
                                o_sb[:],
                            )
    return nc


def _pack_ktiles(a, tile_rows=128):
    """[R, C] -> [128, (R//128)*C] with k-tile kt at cols [kt*C:(kt+1)*C]."""
    r, c = a.shape
    n = r // tile_rows
    return np.ascontiguousarray(
        a.reshape(n, tile_rows, c).transpose(1, 0, 2).reshape(tile_rows, n * c)
    )


def _rope_tables_ds(position_ids):
    """cos/sin in [d, s] layout: [128, Q] f64."""
    pos = np.asarray(position_ids).reshape(-1).astype(np.float64)      # [Q]
    inv_freq = 1.0 / (ROPE_THETA ** (np.arange(0, HD, 2, dtype=np.float64) / HD))
    ang_half = np.outer(inv_freq, pos)                                 # [64, Q]
    ang = np.concatenate([ang_half, ang_half], axis=0)                 # [128, Q]
    return np.cos(ang), np.sin(ang)


def kernel(hidden_states, attention_mask, position_ids, past_k, past_v,
           Wq, Wk, Wv, Wo):
    global LAST_RESULTS
    bf = ml_dtypes.bfloat16

    hs = np.asarray(hidden_states, np.float32).reshape(Q, HID)
    mask = np.asarray(attention_mask, np.float32).reshape(Q, KV)
    cos_d, sin_d = _rope_tables_ds(position_ids)

    scale = 1.0 / math.sqrt(HD)
    cosq_t = (cos_d * scale).astype(bf)
    sinq_t = (sin_d * scale).astype(bf)
    cosk_t = cos_d.astype(bf)
    sink_t = sin_d.astype(bf)

    # rotate-half permutation with sign: rot[d] = -x[d+64] (d<64); x[d-64]
    prot_np = np.zeros((128, 128), np.float32)
    for dd in range(64):
        prot_np[dd + 64, dd] = -1.0     # lhsT[d', d]: rot[d] += P[d', d] * x[d']
        prot_np[dd, dd + 64] = 1.0
    prot_t = prot_np.astype(bf)

    # diagonal masks: [128 kv, 4 tiles * 512 q]: kv tile 24+m vs queries
    # 0..511 (identical pattern to kv tile 28+m vs queries 512..1023)
    mask_t = np.empty((128, 2048), np.float32)
    for m in range(4):
        kt = 24 + m
        blk = mask[0:512, kt * 128:(kt + 1) * 128].T
        mask_t[:, m * 512:(m + 1) * 512] = (blk == 0.0).astype(np.float32)
    mask_t = mask_t.astype(bf)

    sel2_np = np.zeros((2, 256), np.float32)  # cast to bf16 below
    sel2_np[0, 0:128] = 1.0      # sel_a: broadcast rc row 0
    sel2_np[1, 128:256] = 1.0    # sel_b: broadcast rc row 1

    hst = _pack_ktiles(np.ascontiguousarray(hs.T)).astype(bf)      # [128, 32, 1024]
    hst = hst.reshape(128, N_HK, Q)

    nc = _build_program()
    in_maps = []
    for c in range(NCORES):
        qs = slice(c * HPC * HD, (c + 1) * HPC * HD)
        ks = slice(c * HD, (c + 1) * HD)
        wq_c = _pack_ktiles(
            np.ascontiguousarray(Wq[qs, :].T)
        ).astype(bf).reshape(128, N_HK, HPC * 128)
        wk_c = np.ascontiguousarray(Wk[ks, :].T)                   # [4096, 128]
        wv_c = np.ascontiguousarray(Wv[ks, :].T)
        wkv_c = _pack_ktiles(
            np.concatenate([wk_c, wv_c], axis=1)
        ).astype(bf).reshape(128, N_HK, 256)
        pkt = np.ascontiguousarray(past_k[0, c].T).astype(bf)      # [128, 3072]
        pv = _pack_ktiles(np.ascontiguousarray(past_v[0, c])).astype(bf)
        wo_c = _pack_ktiles(
            np.ascontiguousarray(Wo[:, qs].T)).astype(bf)          # [128, 4*4096]
        in_maps.append({
            "hst": hst, "wqt": wq_c, "wkvt": wkv_c, "pastkt": pkt,
            "pastv": pv, "maskt": mask_t, "cosq": cosq_t, "sinq": sinq_t,
            "cosk": cosk_t, "sink": sink_t, "prot": prot_t,
            "sel2": sel2_np.astype(bf),
            "wot": wo_c,
        })

    res = run_bass_kernel_spmd(nc, in_maps, list(range(NCORES)))
    LAST_RESULTS = res
    out = np.zeros((Q, HID), np.float32)
    for c in range(NCORES):
        out += np.asarray(res.results[c]["outp"], dtype=np.float32)
    return out.reshape(B, Q, HID)
